# revision 7
# baseline (speedup 1.0000x reference)
"""Trainium2 Bass kernel for a 2-layer DGCN (graph conv) on 8 NeuronCores.

Reference computation (fp32):
    h1  = relu(IFadj @ (x @ W1) + b1)         # [N, NHID]
    out = BN(adj @ (h1 @ W2) + b2)            # [N, OUTD], BN in eval mode

Distribution: rows of x / IFadj / adj are sharded across 8 cores
(row-parallel graph partitioning). Per core (rows R_k), v3 schedule:

  phase A: S_own = x[R_k] @ W1 (cb-outer over 8 PSUM banks, x/W1
           streamed in 128-row slabs so the PE starts ~1us in).
           The S AllGather is split in two pipelined chunks (half the
           local rows each) fired as soon as their bounce lands
           (~12us / ~15us in) -- the first collective absorbs the
           cross-core launch-skew barrier while local compute runs.
  phase B: redundantly compute S for global node blocks 4..7 (same
           blocks on every core, from a replicated x slice): keeps the
           PE busy through the barrier+gather window so phase C never
           waits on the collective.
  phase C: h1T = relu(S^T @ IFadjT_k + b1), two i-half passes;
           m-traversal order 4..7 (local S) then 0..3 (gathered S,
           chunk-0 quarters before chunk-1 quarters). After each half:
           z-half = h1 @ (W2/4) evicted to fp8 and Z-AllGather chunk
           fired mid-kernel.
  phase D: outT = Z-as-lhsT vs adjT_k rhs, fp8e4 DoubleRow matmuls
           (one instruction covers an adjacent m-tile pair), BN fused
           on the PSUM evict with the x4 range-fold undone in the BN
           scale.

All device DMAs are plain 2D slices: rearranged (strided-permute)
DMAs lower to DMA_DIRECT2D executed serially on the Sync engine and
serialize the whole pipeline behind collective-gated transfers (the
v2 failure mode). The host pre-permutes instead: adj is supplied
pair-interleaved ([pair*128, 2*ROWS]) for DoubleRow, W2 partition-
major, and the z bounce is written p-major so the gathered z is
plain-sliceable.

The PE consumes the left operand transposed (out = lhsT.T @ rhs), so
the host passes IFadj[R_k].T per core; with the h1T/outT formulations
no on-device transposes are needed. Layer-1 matmuls run in bf16; the
layer-2 spmm runs in fp8e4 (empirically ~3e-3 rel err vs the 2e-2
gate; adj entries are U[0,1] and z is scaled by 1/4 so both sit far
inside e4m3 range).
"""

import numpy as np
import ml_dtypes

NCORES = 8
N = 8192
NFEAT = 1024
NHID = 512
OUTD = 256
ROWS = N // NCORES  # 1024
P = 128
BN_EPS = 1e-5

CB = NFEAT // P   # 8  c-blocks (x feature contraction)
IB = ROWS // P    # 8  i-blocks per node block
JB = NHID // P    # 4  j-blocks (hidden)
MT = N // P       # 64 m-tiles (global node contraction)
HF = 512          # matmul moving free dim (PSUM bank limit)
IH = ROWS // HF   # 2 i-halves of the local row range
OB = OUTD // P    # 2 output-feature blocks
GC = 2            # allgather chunks for S and Z
QT = 4            # m-tiles per (chunk, block) quarter
NRED = 4          # redundant S blocks (global blocks 4..7)
RED0 = NCORES - NRED  # first redundant block = 4

_BF16 = ml_dtypes.bfloat16
_F8 = ml_dtypes.float8_e4m3

_cache = {}


def _build():
    import concourse.mybir as mybir
    import concourse.tile as tile
    from concourse import bacc

    dt = mybir.dt
    f32 = dt.float32
    bf16 = dt.bfloat16
    f8 = dt.float8e4
    AF = mybir.ActivationFunctionType
    DR = mybir.MatmulPerfMode.DoubleRow

    nc = bacc.Bacc("TRN2", target_bir_lowering=False, debug=False,
                   num_devices=NCORES)

    xT_e = nc.dram_tensor("xT", [NFEAT, ROWS], bf16, kind="ExternalInput")
    # replicated x rows for global node blocks 4..7 (same on every core)
    xTr_e = nc.dram_tensor("xTr", [NFEAT, NRED * ROWS], bf16,
                           kind="ExternalInput")
    ifadjT_e = nc.dram_tensor("ifadjT", [N, ROWS], bf16, kind="ExternalInput")
    # adj rows pair-interleaved on host: row pair*P+p holds m-tiles
    # (2*pair, 2*pair+1) side by side -> [P, 2, ROWS] is a plain slice
    adjP_e = nc.dram_tensor("adjP", [N // 2, 2 * ROWS], f8,
                            kind="ExternalInput")
    w1_e = nc.dram_tensor("w1", [NFEAT, NHID], bf16, kind="ExternalInput")
    # W2/4, partition-major: [P, JB*OUTD]
    w2_e = nc.dram_tensor("w2", [P, JB * OUTD], bf16, kind="ExternalInput")
    b1p_e = nc.dram_tensor("b1p", [P, JB], f32, kind="ExternalInput")
    bnsc_e = nc.dram_tensor("bnsc", [P, OB], f32, kind="ExternalInput")
    bnbi_e = nc.dram_tensor("bnbi", [P, OB], f32, kind="ExternalInput")
    # outT: [OUTD, ROWS]; the host transposes each core's block.
    out_e = nc.dram_tensor("out", [OUTD, ROWS], f32, kind="ExternalOutput")

    groups = [list(range(NCORES))]

    def allgather(g_in, g_out):
        nc.gpsimd.collective_compute(
            "AllGather", mybir.AluOpType.bypass, replica_groups=groups,
            ins=[g_in[:]], outs=[g_out[:]])

    with tile.TileContext(nc) as tc:
        with (
            tc.tile_pool(name="const", bufs=1) as const,
            tc.tile_pool(name="xslab", bufs=4) as xslab_p,
            tc.tile_pool(name="sloc", bufs=1) as sloc_p,
            tc.tile_pool(name="sred", bufs=1) as sred_p,
            tc.tile_pool(name="sgt", bufs=RED0 * IB) as sgt_p,
            tc.tile_pool(name="h1", bufs=1) as h1_p,
            tc.tile_pool(name="zsb", bufs=1) as z_p,
            tc.tile_pool(name="zchunk", bufs=8) as zchunk_p,
            tc.tile_pool(name="astream", bufs=16) as astream,
            tc.tile_pool(name="apair", bufs=6) as apair_p,
            tc.tile_pool(name="outsb", bufs=1) as outsb_p,
            tc.tile_pool(name="dram", bufs=1, space="DRAM") as dram,
        ):
            # ---- phase-A constants as slab DMAs (first matmul starts
            # once w1 slab 0 + xT slab 0 land, ~1us)
            w1_sb = const.tile([P, CB, NHID], bf16)
            for cb in range(CB):
                nc.sync.dma_start(w1_sb[:, cb, :],
                                  w1_e[cb * P:(cb + 1) * P, :])

            # ---- DRAM bounce buffers for the collectives
            RPC = ROWS // GC  # rows bounced per S chunk (512)
            s_bounce = [dram.tile([RPC, NHID], bf16, name=f"sb{c}")
                        for c in range(GC)]
            # s_all[c] row k*RPC + r = S[global row k*ROWS + c*RPC + r]
            s_all = [dram.tile([RPC * NCORES, NHID], bf16,
                               addr_space="Shared", name=f"sa{c}")
                     for c in range(GC)]
            # z bounce is p-major: row p holds (t, o) for the chunk's 4
            # m-tiles -> gathered z is plain-sliceable per core block
            z_bounce = [dram.tile([P, QT * OUTD], f8, name=f"zb{c}")
                        for c in range(GC)]
            z_all = [dram.tile([P * NCORES, QT * OUTD], f8,
                               addr_space="Shared", name=f"za{c}")
                     for c in range(GC)]

            s_loc = sloc_p.tile([P, IB, NHID], bf16)
            s_red = sred_p.tile([P, NRED * IB, NHID], bf16)

            # ---- phase A: own S block, cb-outer across 8 PSUM banks;
            # bounce per half, fire the two S AllGather chunks early.
            with tc.tile_pool(name="psA", bufs=1, space="PSUM") as psA:
                ps_own = [psA.tile([P, NHID], f32, name=f"pso{ib}",
                                   tag=f"pa{ib}")
                          for ib in range(IB)]
                for cb in range(CB):
                    xs = xslab_p.tile([P, ROWS], bf16, tag="xslab")
                    nc.sync.dma_start(xs[:], xT_e[cb * P:(cb + 1) * P, :])
                    for ib in range(IB):
                        nc.tensor.matmul(
                            ps_own[ib][:], xs[:, ib * P:(ib + 1) * P],
                            w1_sb[:, cb, :],
                            start=(cb == 0), stop=(cb == CB - 1))
                for c in range(GC):
                    for t in range(IB // GC):
                        ib = c * (IB // GC) + t
                        nc.scalar.activation(s_loc[:, ib, :],
                                             ps_own[ib][:], AF.Copy)
                        nc.sync.dma_start(
                            s_bounce[c][t * P:(t + 1) * P, :],
                            s_loc[:, ib, :])
                    allgather(s_bounce[c], s_all[c])

                # remaining constants (needed from phase C on)
                b1p_sb = const.tile([P, JB], f32)
                nc.sync.dma_start(b1p_sb[:], b1p_e[:])
                w2_sb = const.tile([P, JB, OUTD], bf16)
                nc.sync.dma_start(w2_sb[:], w2_e[:])
                bnsc_sb = const.tile([P, OB], f32)
                nc.sync.dma_start(bnsc_sb[:], bnsc_e[:])
                bnbi_sb = const.tile([P, OB], f32)
                nc.sync.dma_start(bnbi_sb[:], bnbi_e[:])

                # ---- phase B: redundant S for global blocks 4..7
                for r in range(NRED):
                    ps_r = [psA.tile([P, NHID], f32, name=f"psr{r}_{ib}",
                                     tag=f"pa{ib}")
                            for ib in range(IB)]
                    for cb in range(CB):
                        xs = xslab_p.tile([P, ROWS], bf16, tag="xslab")
                        nc.sync.dma_start(
                            xs[:],
                            xTr_e[cb * P:(cb + 1) * P,
                                  r * ROWS:(r + 1) * ROWS])
                        for ib in range(IB):
                            nc.tensor.matmul(
                                ps_r[ib][:], xs[:, ib * P:(ib + 1) * P],
                                w1_sb[:, cb, :],
                                start=(cb == 0), stop=(cb == CB - 1))
                    for ib in range(IB):
                        nc.scalar.activation(
                            s_red[:, r * IB + ib, :], ps_r[ib][:], AF.Copy)

            h1T = h1_p.tile([P, JB, ROWS], bf16)
            z_sb = z_p.tile([P, IB, OUTD], f8)
            # gathered-S tiles: per (block g<RED0, q); persist across both
            # i-half passes. s_all[c] slice for (g, q): rows g*RPC + qq*P.
            s_gt = [[None] * IB for _ in range(RED0)]

            # m-traversal: local blocks first, then gathered blocks with
            # chunk-0 quarters (q 0..3) before chunk-1 quarters (q 4..7).
            walk = ([(g, q) for g in range(RED0, NCORES) for q in range(IB)]
                    + [(g, c * QT + t) for c in range(GC)
                       for g in range(RED0) for t in range(QT)])

            # ---- phase C, i-half pass ih: accumulate h1T half over all
            # 64 m-tiles, evict relu half, emit z half fp8, fire Z chunk.
            def l1_pass(ih, psh, psz):
                psum_h = [psh.tile([P, HF], f32, name=f"ph{jb}_{ih}",
                                   tag=f"ph{jb}")
                          for jb in range(JB)]
                for n_emitted, (g, q) in enumerate(walk):
                    if g >= RED0:
                        s_src = s_red[:, (g - RED0) * IB + q, :]
                    else:
                        if ih == 0 and s_gt[g][q] is None:
                            c, t = divmod(q, QT)
                            st = sgt_p.tile([P, NHID], bf16,
                                            name=f"sg{g}_{q}", tag="sgt")
                            nc.sync.dma_start(
                                st[:],
                                s_all[c][(g * QT + t) * P:
                                         (g * QT + t + 1) * P, :])
                            s_gt[g][q] = st
                        s_src = s_gt[g][q][:]
                    mt = g * IB + q
                    a_tile = astream.tile([P, HF], bf16, tag="ahalf")
                    nc.sync.dma_start(
                        a_tile[:],
                        ifadjT_e[mt * P:(mt + 1) * P,
                                 ih * HF:(ih + 1) * HF])
                    for jb in range(JB):
                        nc.tensor.matmul(
                            psum_h[jb][:],
                            s_src[:, jb * P:(jb + 1) * P],
                            a_tile[:],
                            start=(n_emitted == 0),
                            stop=(n_emitted == MT - 1),
                        )
                # epilogue: relu+bias into h1T half
                for jb in range(JB):
                    nc.scalar.activation(
                        h1T[:, jb, ih * HF:(ih + 1) * HF],
                        psum_h[jb][:], AF.Relu,
                        bias=b1p_sb[:, jb:jb + 1])
                # z for this half's i-blocks (fp8, W2 pre-scaled by 1/4),
                # p-major bounce, gather chunk ih
                for t in range(IB // IH):
                    ib = ih * (IB // IH) + t
                    ps = psz.tile([P, OUTD], f32, tag="z")
                    for jb in range(JB):
                        nc.tensor.matmul(
                            ps[:],
                            h1T[:, jb, ib * P:(ib + 1) * P],
                            w2_sb[:, jb, :],
                            start=(jb == 0), stop=(jb == JB - 1),
                        )
                    nc.scalar.activation(z_sb[:, ib, :], ps[:], AF.Copy)
                    nc.sync.dma_start(
                        z_bounce[ih][:, t * OUTD:(t + 1) * OUTD],
                        z_sb[:, ib, :])
                allgather(z_bounce[ih], z_all[ih])

            with (
                tc.tile_pool(name="psh", bufs=1, space="PSUM") as psh,
                tc.tile_pool(name="psz", bufs=2, space="PSUM") as psz,
            ):
                for ih in range(IH):
                    l1_pass(ih, psh, psz)

            # ---- phase D: outT[o, i] = sum_m Z[m, o] * adjT[m, i]
            # fp8 DoubleRow: one matmul covers an adjacent m-tile pair.
            # z_all[c] row k*P+p holds (t, o) = z[k*ROWS + c*RPC + t*P + p]
            # -> m-tile of (c, k, t) is 8k + 4c + t.
            outT_sb = outsb_p.tile([P, OB, ROWS], f32)
            with tc.tile_pool(name="ps4", bufs=1, space="PSUM") as ps4:
                psum_o = [[ps4.tile([P, HF], f32, name=f"po{ob}_{ih}",
                                    tag=f"po{ob}_{ih}")
                           for ih in range(IH)] for ob in range(OB)]
                first = True
                for c in range(GC):
                    for k in range(NCORES):
                        zc_sb = zchunk_p.tile([P, QT, OUTD], f8,
                                              tag="zchunk")
                        nc.sync.dma_start(
                            zc_sb[:], z_all[c][k * P:(k + 1) * P, :])
                        last_grp = (c == GC - 1 and k == NCORES - 1)
                        for pr in range(0, QT, 2):
                            mt = IB * k + QT * c + pr
                            a_pair = apair_p.tile([P, 2, ROWS], f8,
                                                  tag="apair")
                            nc.sync.dma_start(
                                a_pair[:],
                                adjP_e[(mt // 2) * P:(mt // 2 + 1) * P, :])
                            last_pr = last_grp and pr == QT - 2
                            for ob in range(OB):
                                for ih in range(IH):
                                    nc.tensor.matmul(
                                        psum_o[ob][ih][:],
                                        zc_sb[:, pr:pr + 2,
                                              ob * P:(ob + 1) * P],
                                        a_pair[:, :,
                                               ih * HF:(ih + 1) * HF],
                                        start=first, stop=last_pr,
                                        perf_mode=DR,
                                    )
                            first = False
                # fused BN affine on PSUM evict: out = psum*scale + bias
                for ob in range(OB):
                    for ih in range(IH):
                        nc.vector.tensor_scalar(
                            outT_sb[:, ob, ih * HF:(ih + 1) * HF],
                            psum_o[ob][ih][:],
                            bnsc_sb[:, ob:ob + 1],
                            bnbi_sb[:, ob:ob + 1],
                            mybir.AluOpType.mult,
                            mybir.AluOpType.add)
                    nc.sync.dma_start(
                        out_e[ob * P:(ob + 1) * P, :], outT_sb[:, ob, :])

    nc.compile()
    return nc


def _get_nc():
    if "nc" not in _cache:
        _cache["nc"] = _build()
    return _cache["nc"]


def kernel(x, IFadj, adj, W1, b1, W2, b2, bn_gamma, bn_beta, bn_mean, bn_var):
    from concourse.bass_utils import run_bass_kernel_spmd

    x = np.asarray(x, dtype=np.float32)
    IFadj = np.asarray(IFadj, dtype=np.float32)
    adj = np.asarray(adj, dtype=np.float32)
    W1 = np.asarray(W1, dtype=np.float32)
    b1 = np.asarray(b1, dtype=np.float32)
    W2 = np.asarray(W2, dtype=np.float32)
    b2 = np.asarray(b2, dtype=np.float32)
    bn_gamma = np.asarray(bn_gamma, dtype=np.float32)
    bn_beta = np.asarray(bn_beta, dtype=np.float32)
    bn_mean = np.asarray(bn_mean, dtype=np.float32)
    bn_var = np.asarray(bn_var, dtype=np.float32)

    # host-side prep: shard rows, transpose for PE lhsT layout, cast.
    # W2 is pre-scaled by 1/4 so z stays well inside fp8e4 range; the
    # BN scale is multiplied by 4 to undo it after the layer-2 spmm.
    w1b = W1.astype(_BF16)
    # W2/4 partition-major: [P, JB*OUTD]
    w2b = np.ascontiguousarray(
        (W2 * 0.25).astype(_BF16).reshape(JB, P, OUTD)
        .transpose(1, 0, 2).reshape(P, JB * OUTD))
    b1p = np.ascontiguousarray(b1.reshape(JB, P).T)  # [P, JB]
    inv = bn_gamma / np.sqrt(bn_var + BN_EPS)
    bias_tot = b2 * inv + bn_beta - bn_mean * inv
    bnsc = np.ascontiguousarray((4.0 * inv).reshape(OB, P).T)   # [P, OB]
    bnbi = np.ascontiguousarray(bias_tot.reshape(OB, P).T)      # [P, OB]

    # replicated x rows for global node blocks 4..7
    xTr = np.ascontiguousarray(x[RED0 * ROWS:].T).astype(_BF16)

    in_maps = []
    for k in range(NCORES):
        r0, r1 = k * ROWS, (k + 1) * ROWS
        adjT8 = np.ascontiguousarray(adj[r0:r1].T).astype(_F8)  # [N, ROWS]
        # pair-interleave: row pair*P+p = m-tiles (2p, 2p+1) side by side
        adjP = np.ascontiguousarray(
            adjT8.reshape(N // 256, 2, P, ROWS).transpose(0, 2, 1, 3)
            .reshape(N // 2, 2 * ROWS))
        in_maps.append({
            "xT": np.ascontiguousarray(x[r0:r1].T).astype(_BF16),
            "xTr": xTr,
            "ifadjT": np.ascontiguousarray(IFadj[r0:r1].T).astype(_BF16),
            "adjP": adjP,
            "w1": w1b,
            "w2": w2b,
            "b1p": b1p,
            "bnsc": bnsc,
            "bnbi": bnbi,
        })

    global _last_in_maps
    _last_in_maps = in_maps

    nc = _get_nc()
    try:
        res = run_bass_kernel_spmd(nc, in_maps, list(range(NCORES)))
    except Exception:
        # transient device wedge (NRT_EXEC_UNIT_UNRECOVERABLE etc.) --
        # a straight retry has been observed to recover
        import time
        time.sleep(2.0)
        res = run_bass_kernel_spmd(nc, in_maps, list(range(NCORES)))
    # per-core output is outT [OUTD, ROWS]; transpose back and stack rows
    return np.concatenate(
        [np.ascontiguousarray(res.results[k]["out"].T)
         for k in range(NCORES)], axis=0)


# revision 9
# speedup vs baseline: 1.0077x; 1.0077x over previous
"""Trainium2 Bass kernel for a 2-layer DGCN (graph conv) on 8 NeuronCores.

Reference computation (fp32):
    h1  = relu(IFadj @ (x @ W1) + b1)         # [N, NHID]
    out = BN(adj @ (h1 @ W2) + b2)            # [N, OUTD], BN in eval mode

Distribution: rows of x / IFadj / adj are sharded across 8 cores
(row-parallel graph partitioning). Per core (rows R_k), v4 schedule:

  phase A: S_own = x[R_k] @ W1 (cb-outer over 8 PSUM banks, x/W1
           streamed in 128-row slabs); two pipelined S-AllGather
           chunks fired as soon as their bounce lands. The first
           collective absorbs the cross-core launch-skew barrier
           (~45us, unavoidable) while local compute keeps running.
  phase B: redundantly compute S for global node blocks 5..7 (same on
           every core, from a replicated x slice).
  phase C: h1T accumulation, BOTH row-halves at once across all 8
           PSUM banks, in two strictly sequential sub-phases:
             C1 = local m-blocks 5..7 (no collective dependency),
             C2 = gathered m-blocks 0..4 (chunk-0 quarters first).
           All AllGather-gated DMAs are issued at the C2 boundary:
           every tile-framework DMA executes serially in program
           order on the Sync engine, so a gated DMA ahead of
           independent work stalls the whole pipeline (v2/v3 failure
           mode). Then relu-evict, z = h1 @ (W2/4) in fp8, ONE
           Z-AllGather.
  phase D: outT = Z-as-lhsT vs adjT_k rhs, fp8e4 DoubleRow matmuls
           (one instruction per adjacent m-tile pair); the adj pair
           tiles are prefetched BEFORE the AllGather-gated z loads so
           the gather wait cannot block them; BN fused on the PSUM
           evict with the x4 range-fold undone in the BN scale.

Precision: layer-1 runs lhsT=S in bf16 against the IFadj moving
operand in fp8e4, CENTERED: B = IFadj - 1/2 (entries U[0,1] ->
[-1/2,1/2]), and the exact mean term  1/2 * colsum(S)_j  is folded
into the layer-1 bias on the host (colsum(S) = (sum_m x[m,:]) @ W1,
a trivial host matvec). This halves the fp8 quantization noise and
makes the dominant mean component exact: measured end-to-end rel err
~3.5e-3 vs the 2e-2 gate. The layer-2 spmm runs both operands fp8e4
with z pre-scaled by 1/4 to sit far inside e4m3 range.

The PE consumes the left operand transposed (out = lhsT.T @ rhs); the
host passes IFadj[R_k].T / adj pair-interleaved per core, W2
partition-major, so no on-device transposes or strided-permute DMAs
are needed anywhere (those lower to serialized Sync-engine transfers).
"""

import numpy as np
import ml_dtypes

NCORES = 8
N = 8192
NFEAT = 1024
NHID = 512
OUTD = 256
ROWS = N // NCORES  # 1024
P = 128
BN_EPS = 1e-5

CB = NFEAT // P   # 8  c-blocks (x feature contraction)
IB = ROWS // P    # 8  i-blocks per node block
JB = NHID // P    # 4  j-blocks (hidden)
MT = N // P       # 64 m-tiles (global node contraction)
HF = 512          # matmul moving free dim (PSUM bank limit)
IH = ROWS // HF   # 2 i-halves of the local row range
OB = OUTD // P    # 2 output-feature blocks
GC = 2            # S allgather chunks
QT = 4            # m-tiles per (chunk, block) quarter
NRED = 3          # redundant S blocks (global blocks 5..7)
RED0 = NCORES - NRED  # first redundant block = 5

_BF16 = ml_dtypes.bfloat16
_F8 = ml_dtypes.float8_e4m3

_cache = {}


def _build():
    import concourse.mybir as mybir
    import concourse.tile as tile
    from concourse import bacc

    dt = mybir.dt
    f32 = dt.float32
    bf16 = dt.bfloat16
    f8 = dt.float8e4
    AF = mybir.ActivationFunctionType
    DR = mybir.MatmulPerfMode.DoubleRow

    nc = bacc.Bacc("TRN2", target_bir_lowering=False, debug=False,
                   num_devices=NCORES)

    xT_e = nc.dram_tensor("xT", [NFEAT, ROWS], bf16, kind="ExternalInput")
    # replicated x rows for global node blocks 5..7 (same on every core)
    xTr_e = nc.dram_tensor("xTr", [NFEAT, NRED * ROWS], bf16,
                           kind="ExternalInput")
    # centered IFadj^T in fp8: entries IFadj - 1/2
    ifadjT_e = nc.dram_tensor("ifadjT", [N, ROWS], f8, kind="ExternalInput")
    # adj rows pair-interleaved on host: row pair*P+p holds m-tiles
    # (2*pair, 2*pair+1) side by side -> [P, 2, ROWS] is a plain slice
    adjP_e = nc.dram_tensor("adjP", [N // 2, 2 * ROWS], f8,
                            kind="ExternalInput")
    w1_e = nc.dram_tensor("w1", [NFEAT, NHID], bf16, kind="ExternalInput")
    # W2/4, partition-major: [P, JB*OUTD]
    w2_e = nc.dram_tensor("w2", [P, JB * OUTD], bf16, kind="ExternalInput")
    # layer-1 bias + 1/2*colsum(S) fold, [P, JB]
    b1p_e = nc.dram_tensor("b1p", [P, JB], f32, kind="ExternalInput")
    bnsc_e = nc.dram_tensor("bnsc", [P, OB], f32, kind="ExternalInput")
    bnbi_e = nc.dram_tensor("bnbi", [P, OB], f32, kind="ExternalInput")
    # outT: [OUTD, ROWS]; the host transposes each core's block.
    out_e = nc.dram_tensor("out", [OUTD, ROWS], f32, kind="ExternalOutput")

    groups = [list(range(NCORES))]

    def allgather(g_in, g_out):
        nc.gpsimd.collective_compute(
            "AllGather", mybir.AluOpType.bypass, replica_groups=groups,
            ins=[g_in[:]], outs=[g_out[:]])

    with tile.TileContext(nc) as tc:
        with (
            tc.tile_pool(name="const", bufs=1) as const,
            tc.tile_pool(name="xslab", bufs=4) as xslab_p,
            tc.tile_pool(name="sloc", bufs=1) as sloc_p,
            tc.tile_pool(name="sred", bufs=1) as sred_p,
            tc.tile_pool(name="sgt", bufs=(NCORES - NRED) * IB) as sgt_p,
            tc.tile_pool(name="h1", bufs=1) as h1_p,
            tc.tile_pool(name="zsb", bufs=1) as z_p,
            tc.tile_pool(name="zchunk", bufs=8) as zchunk_p,
            tc.tile_pool(name="afull", bufs=16) as afull_p,
            tc.tile_pool(name="apair", bufs=12) as apair_p,
            tc.tile_pool(name="outsb", bufs=1) as outsb_p,
            tc.tile_pool(name="dram", bufs=1, space="DRAM") as dram,
        ):
            # ---- phase-A constants as slab DMAs (first matmul starts
            # once w1 slab 0 + xT slab 0 land)
            w1_sb = const.tile([P, CB, NHID], bf16)
            for cb in range(CB):
                nc.sync.dma_start(w1_sb[:, cb, :],
                                  w1_e[cb * P:(cb + 1) * P, :])

            # ---- DRAM bounce buffers for the collectives
            RPC = ROWS // GC  # rows bounced per S chunk (512)
            s_bounce = [dram.tile([RPC, NHID], bf16, name=f"sb{c}")
                        for c in range(GC)]
            # s_all[c] row k*RPC + r = S[global row k*ROWS + c*RPC + r]
            s_all = [dram.tile([RPC * NCORES, NHID], bf16,
                               addr_space="Shared", name=f"sa{c}")
                     for c in range(GC)]
            # z bounce is p-major: row p holds (t, o), t = local i-block
            z_bounce = dram.tile([P, IB * OUTD], f8, name="zb")
            z_all = dram.tile([P * NCORES, IB * OUTD], f8,
                              addr_space="Shared", name="za")

            s_loc = sloc_p.tile([P, IB, NHID], bf16)
            s_red = sred_p.tile([P, NRED * IB, NHID], bf16)

            # ---- phase A: own S block, cb-outer across 8 PSUM banks;
            # bounce per half, fire the two S AllGather chunks early.
            with tc.tile_pool(name="psA", bufs=1, space="PSUM") as psA:
                ps_own = [psA.tile([P, NHID], f32, name=f"pso{ib}",
                                   tag=f"pa{ib}")
                          for ib in range(IB)]
                for cb in range(CB):
                    xs = xslab_p.tile([P, ROWS], bf16, tag="xslab")
                    nc.sync.dma_start(xs[:], xT_e[cb * P:(cb + 1) * P, :])
                    for ib in range(IB):
                        nc.tensor.matmul(
                            ps_own[ib][:], xs[:, ib * P:(ib + 1) * P],
                            w1_sb[:, cb, :],
                            start=(cb == 0), stop=(cb == CB - 1))
                for c in range(GC):
                    for t in range(IB // GC):
                        ib = c * (IB // GC) + t
                        nc.scalar.activation(s_loc[:, ib, :],
                                             ps_own[ib][:], AF.Copy)
                        nc.sync.dma_start(
                            s_bounce[c][t * P:(t + 1) * P, :],
                            s_loc[:, ib, :])
                    allgather(s_bounce[c], s_all[c])

                # remaining constants (needed from phase C on)
                b1p_sb = const.tile([P, JB], f32)
                nc.sync.dma_start(b1p_sb[:], b1p_e[:])
                w2_sb = const.tile([P, JB, OUTD], bf16)
                nc.sync.dma_start(w2_sb[:], w2_e[:])
                bnsc_sb = const.tile([P, OB], f32)
                nc.sync.dma_start(bnsc_sb[:], bnsc_e[:])
                bnbi_sb = const.tile([P, OB], f32)
                nc.sync.dma_start(bnbi_sb[:], bnbi_e[:])

                # ---- phase B: redundant S for global blocks 5..7
                for r in range(NRED):
                    ps_r = [psA.tile([P, NHID], f32, name=f"psr{r}_{ib}",
                                     tag=f"pa{ib}")
                            for ib in range(IB)]
                    for cb in range(CB):
                        xs = xslab_p.tile([P, ROWS], bf16, tag="xslab")
                        nc.sync.dma_start(
                            xs[:],
                            xTr_e[cb * P:(cb + 1) * P,
                                  r * ROWS:(r + 1) * ROWS])
                        for ib in range(IB):
                            nc.tensor.matmul(
                                ps_r[ib][:], xs[:, ib * P:(ib + 1) * P],
                                w1_sb[:, cb, :],
                                start=(cb == 0), stop=(cb == CB - 1))
                    for ib in range(IB):
                        nc.scalar.activation(
                            s_red[:, r * IB + ib, :], ps_r[ib][:], AF.Copy)

            h1T = h1_p.tile([P, JB, ROWS], bf16)
            z_sb = z_p.tile([P, IB, OUTD], f8)

            # ---- phase C: h1T += S[m]^T @ B[m] over all 64 m-tiles,
            # both row-halves at once (8 PSUM banks). C1 = local blocks
            # (5..7), C2 = gathered blocks (0..4, chunk-0 quarters first).
            with tc.tile_pool(name="psC", bufs=1, space="PSUM") as psC:
                psum_h = [[psC.tile([P, HF], f32, name=f"ph{jb}_{ih}",
                                    tag=f"ph{jb}_{ih}")
                           for ih in range(IH)] for jb in range(JB)]

                def c_mms(s_src, mt, first, last):
                    a_full = afull_p.tile([P, ROWS], f8, tag="afull")
                    nc.sync.dma_start(
                        a_full[:], ifadjT_e[mt * P:(mt + 1) * P, :])
                    for jb in range(JB):
                        for ih in range(IH):
                            nc.tensor.matmul(
                                psum_h[jb][ih][:],
                                s_src[:, jb * P:(jb + 1) * P],
                                a_full[:, ih * HF:(ih + 1) * HF],
                                start=first, stop=last)

                # C1: local blocks
                first = True
                for g in range(RED0, NCORES):
                    for q in range(IB):
                        c_mms(s_red[:, (g - RED0) * IB + q, :],
                              g * IB + q, first, False)
                        first = False
                # C2: gathered blocks; stage tiles issued chunk-major at
                # the boundary, then consumed in the same order
                s_gt = {}
                for c in range(GC):
                    for g in range(RED0):
                        for t in range(QT):
                            st = sgt_p.tile([P, NHID], bf16,
                                            name=f"sg{c}_{g}_{t}",
                                            tag="sgt")
                            nc.sync.dma_start(
                                st[:],
                                s_all[c][(g * QT + t) * P:
                                         (g * QT + t + 1) * P, :])
                            s_gt[(c, g, t)] = st
                for c in range(GC):
                    for g in range(RED0):
                        for t in range(QT):
                            c_mms(s_gt[(c, g, t)][:],
                                  g * IB + c * QT + t, False,
                                  c == GC - 1 and g == RED0 - 1
                                  and t == QT - 1)
                # relu + (b1 + colsum/2) bias into h1T
                for jb in range(JB):
                    for ih in range(IH):
                        nc.scalar.activation(
                            h1T[:, jb, ih * HF:(ih + 1) * HF],
                            psum_h[jb][ih][:], AF.Relu,
                            bias=b1p_sb[:, jb:jb + 1])

            # ---- z = h1 @ (W2/4) in fp8, p-major bounce, one AllGather
            with tc.tile_pool(name="psz", bufs=2, space="PSUM") as psz:
                for ib in range(IB):
                    ps = psz.tile([P, OUTD], f32, tag="z")
                    for jb in range(JB):
                        nc.tensor.matmul(
                            ps[:],
                            h1T[:, jb, ib * P:(ib + 1) * P],
                            w2_sb[:, jb, :],
                            start=(jb == 0), stop=(jb == JB - 1),
                        )
                    nc.scalar.activation(z_sb[:, ib, :], ps[:], AF.Copy)
                    nc.sync.dma_start(
                        z_bounce[:, ib * OUTD:(ib + 1) * OUTD],
                        z_sb[:, ib, :])
                allgather(z_bounce, z_all)

            # ---- phase D: outT[o, i] = sum_m Z[m, o] * adjT[m, i]
            # fp8 DoubleRow, one matmul per adjacent m-tile pair.
            # z_all row k*P+p holds (t, o) = z[k*ROWS + t*P + p]
            # -> m-tile of (k, t) is 8k + t; pairs are (t, t+1), t even.
            outT_sb = outsb_p.tile([P, OB, ROWS], f32)
            with tc.tile_pool(name="ps4", bufs=1, space="PSUM") as ps4:
                # prefetch ALL adj pair tiles before the gather-gated z
                # loads so the Z-AllGather wait cannot block them
                a_pairs = []
                for k in range(NCORES):
                    for pr in range(0, IB, 2):
                        ap = apair_p.tile([P, 2, ROWS], f8,
                                          name=f"ap{k}_{pr}", tag="apair")
                        pidx = (IB * k + pr) // 2
                        nc.sync.dma_start(
                            ap[:], adjP_e[pidx * P:(pidx + 1) * P, :])
                        a_pairs.append(ap)
                psum_o = [[ps4.tile([P, HF], f32, name=f"po{ob}_{ih}",
                                    tag=f"po{ob}_{ih}")
                           for ih in range(IH)] for ob in range(OB)]
                first = True
                napc = 0
                for k in range(NCORES):
                    zc_sb = zchunk_p.tile([P, IB, OUTD], f8, tag="zchunk")
                    nc.sync.dma_start(
                        zc_sb[:], z_all[k * P:(k + 1) * P, :])
                    for pr in range(0, IB, 2):
                        a_pair = a_pairs[napc]
                        napc += 1
                        last_pr = (k == NCORES - 1 and pr == IB - 2)
                        for ob in range(OB):
                            for ih in range(IH):
                                nc.tensor.matmul(
                                    psum_o[ob][ih][:],
                                    zc_sb[:, pr:pr + 2,
                                          ob * P:(ob + 1) * P],
                                    a_pair[:, :, ih * HF:(ih + 1) * HF],
                                    start=first, stop=last_pr,
                                    perf_mode=DR,
                                )
                        first = False
                # fused BN affine on PSUM evict: out = psum*scale + bias
                for ob in range(OB):
                    for ih in range(IH):
                        nc.vector.tensor_scalar(
                            outT_sb[:, ob, ih * HF:(ih + 1) * HF],
                            psum_o[ob][ih][:],
                            bnsc_sb[:, ob:ob + 1],
                            bnbi_sb[:, ob:ob + 1],
                            mybir.AluOpType.mult,
                            mybir.AluOpType.add)
                    nc.sync.dma_start(
                        out_e[ob * P:(ob + 1) * P, :], outT_sb[:, ob, :])

    nc.compile()
    return nc


def _get_nc():
    if "nc" not in _cache:
        _cache["nc"] = _build()
    return _cache["nc"]


def kernel(x, IFadj, adj, W1, b1, W2, b2, bn_gamma, bn_beta, bn_mean, bn_var):
    from concourse.bass_utils import run_bass_kernel_spmd

    x = np.asarray(x, dtype=np.float32)
    IFadj = np.asarray(IFadj, dtype=np.float32)
    adj = np.asarray(adj, dtype=np.float32)
    W1 = np.asarray(W1, dtype=np.float32)
    b1 = np.asarray(b1, dtype=np.float32)
    W2 = np.asarray(W2, dtype=np.float32)
    b2 = np.asarray(b2, dtype=np.float32)
    bn_gamma = np.asarray(bn_gamma, dtype=np.float32)
    bn_beta = np.asarray(bn_beta, dtype=np.float32)
    bn_mean = np.asarray(bn_mean, dtype=np.float32)
    bn_var = np.asarray(bn_var, dtype=np.float32)

    # host-side prep: shard rows, transpose for PE lhsT layout, cast.
    # W2 is pre-scaled by 1/4 so z stays well inside fp8e4 range; the
    # BN scale is multiplied by 4 to undo it after the layer-2 spmm.
    w1b = W1.astype(_BF16)
    w2b = np.ascontiguousarray(
        (W2 * 0.25).astype(_BF16).reshape(JB, P, OUTD)
        .transpose(1, 0, 2).reshape(P, JB * OUTD))
    # layer-1 bias including the exact 1/2*colsum(S) centering term
    colsum = x.sum(axis=0, dtype=np.float64).astype(np.float32) @ W1
    b1c = b1 + 0.5 * colsum
    b1p = np.ascontiguousarray(b1c.reshape(JB, P).T)  # [P, JB]
    inv = bn_gamma / np.sqrt(bn_var + BN_EPS)
    bias_tot = b2 * inv + bn_beta - bn_mean * inv
    bnsc = np.ascontiguousarray((4.0 * inv).reshape(OB, P).T)   # [P, OB]
    bnbi = np.ascontiguousarray(bias_tot.reshape(OB, P).T)      # [P, OB]

    # replicated x rows for global node blocks 5..7
    xTr = np.ascontiguousarray(x[RED0 * ROWS:].T).astype(_BF16)

    in_maps = []
    for k in range(NCORES):
        r0, r1 = k * ROWS, (k + 1) * ROWS
        adjT8 = np.ascontiguousarray(adj[r0:r1].T).astype(_F8)  # [N, ROWS]
        # pair-interleave: row pair*P+p = m-tiles (2p, 2p+1) side by side
        adjP = np.ascontiguousarray(
            adjT8.reshape(N // 256, 2, P, ROWS).transpose(0, 2, 1, 3)
            .reshape(N // 2, 2 * ROWS))
        in_maps.append({
            "xT": np.ascontiguousarray(x[r0:r1].T).astype(_BF16),
            "xTr": xTr,
            "ifadjT": np.ascontiguousarray(
                IFadj[r0:r1].T - np.float32(0.5)).astype(_F8),
            "adjP": adjP,
            "w1": w1b,
            "w2": w2b,
            "b1p": b1p,
            "bnsc": bnsc,
            "bnbi": bnbi,
        })

    global _last_in_maps
    _last_in_maps = in_maps

    nc = _get_nc()
    try:
        res = run_bass_kernel_spmd(nc, in_maps, list(range(NCORES)))
    except Exception:
        # transient device wedge (NRT_EXEC_UNIT_UNRECOVERABLE etc.) --
        # a straight retry has been observed to recover
        import time
        time.sleep(2.0)
        res = run_bass_kernel_spmd(nc, in_maps, list(range(NCORES)))
    # per-core output is outT [OUTD, ROWS]; transpose back and stack rows
    return np.concatenate(
        [np.ascontiguousarray(res.results[k]["out"].T)
         for k in range(NCORES)], axis=0)


# revision 10
# speedup vs baseline: 1.0535x; 1.0454x over previous
"""Trainium2 Bass kernel for a 2-layer DGCN (graph conv) on 8 NeuronCores.

Reference computation (fp32):
    h1  = relu(IFadj @ (x @ W1) + b1)         # [N, NHID]
    out = BN(adj @ (h1 @ W2) + b2)            # [N, OUTD], BN in eval mode

Distribution: rows of x / IFadj / adj are sharded across 8 cores
(row-parallel graph partitioning). Per core (rows R_k), v5 schedule:

  phase A: S_own = x[R_k] @ W1 (cb-outer over 8 PSUM banks, x/W1
           streamed in 128-row slabs); two pipelined S-AllGather
           chunks fired as soon as their bounce lands. The first
           collective absorbs the cross-core launch-skew barrier
           (~45us, unavoidable) while local compute keeps running.
  phase B: redundantly compute S for global node blocks 5..7 (same on
           every core, from a replicated x slice): local work that
           covers the barrier + gather window.
  phase C: h1T = relu(S^T @ B + bias), two i-half passes; within each
           pass, local m-blocks 5..7 first (no collective dep), then
           gathered blocks 0..4 chunk-major. After each pass: z-half
           = h1 @ (W2/4) in fp8, Z-AllGather chunk fired mid-kernel.
  phase D: outT = Z-as-lhsT vs adjT_k rhs, fp8e4 DoubleRow matmuls
           (one instruction per adjacent m-tile pair); BN fused on
           the PSUM evict with the x4 range-fold undone in the BN
           scale.

DMA-queue discipline (the v2-v4 lesson): every dma_start executes
serially, in program order, on its issuing engine's HW-DGE queue; a
collective-gated DMA at the queue head blocks everything behind it.
TRN2 exposes TWO queues (SP via nc.sync, Activation via nc.scalar).
The big streaming loads (ifadjT tiles, adj pair tiles) go on the
Activation queue; setup + collective-gated staging (x/W1 slabs,
bounces, gathered-S tiles, gathered-z tiles) go on the SP queue. All
transfers are plain 2D slices -- strided-permute DMAs are avoided
entirely (host pre-permutes adj pair-interleaved and W2
partition-major; the z bounce is written p-major).

Precision: layer-1 runs lhsT=S in bf16 against the IFadj moving
operand in fp8e4, CENTERED: B = IFadj - 1/2 (entries U[0,1] ->
[-1/2,1/2]); the exact mean term 1/2*colsum(S)_j is folded into the
layer-1 bias on the host (colsum(S) = (sum_m x[m,:]) @ W1, a trivial
host matvec). Measured end-to-end rel err ~2.2e-3 vs the 2e-2 gate.
The layer-2 spmm runs both operands fp8e4 with z pre-scaled by 1/4 to
sit far inside e4m3 range.
"""

import numpy as np
import ml_dtypes

NCORES = 8
N = 8192
NFEAT = 1024
NHID = 512
OUTD = 256
ROWS = N // NCORES  # 1024
P = 128
BN_EPS = 1e-5

CB = NFEAT // P   # 8  c-blocks (x feature contraction)
IB = ROWS // P    # 8  i-blocks per node block
JB = NHID // P    # 4  j-blocks (hidden)
MT = N // P       # 64 m-tiles (global node contraction)
HF = 512          # matmul moving free dim (PSUM bank limit)
IH = ROWS // HF   # 2 i-halves of the local row range
OB = OUTD // P    # 2 output-feature blocks
GC = 2            # allgather chunks (S and Z)
QT = 4            # m-tiles per (chunk, block) quarter
NRED = 3          # redundant S blocks (global blocks 5..7)
RED0 = NCORES - NRED  # first redundant block = 5

_BF16 = ml_dtypes.bfloat16
_F8 = ml_dtypes.float8_e4m3

_cache = {}


def _build():
    import concourse.mybir as mybir
    import concourse.tile as tile
    from concourse import bacc

    dt = mybir.dt
    f32 = dt.float32
    bf16 = dt.bfloat16
    f8 = dt.float8e4
    AF = mybir.ActivationFunctionType
    DR = mybir.MatmulPerfMode.DoubleRow

    nc = bacc.Bacc("TRN2", target_bir_lowering=False, debug=False,
                   num_devices=NCORES)

    xT_e = nc.dram_tensor("xT", [NFEAT, ROWS], bf16, kind="ExternalInput")
    # replicated x rows for global node blocks 5..7 (same on every core)
    xTr_e = nc.dram_tensor("xTr", [NFEAT, NRED * ROWS], bf16,
                           kind="ExternalInput")
    # centered IFadj^T in fp8: entries IFadj - 1/2
    ifadjT_e = nc.dram_tensor("ifadjT", [N, ROWS], f8, kind="ExternalInput")
    # adj rows pair-interleaved on host: row pair*P+p holds m-tiles
    # (2*pair, 2*pair+1) side by side -> [P, 2, ROWS] is a plain slice
    adjP_e = nc.dram_tensor("adjP", [N // 2, 2 * ROWS], f8,
                            kind="ExternalInput")
    w1_e = nc.dram_tensor("w1", [NFEAT, NHID], bf16, kind="ExternalInput")
    # W2/4, partition-major: [P, JB*OUTD]
    w2_e = nc.dram_tensor("w2", [P, JB * OUTD], bf16, kind="ExternalInput")
    # layer-1 bias + 1/2*colsum(S) fold, [P, JB]
    b1p_e = nc.dram_tensor("b1p", [P, JB], f32, kind="ExternalInput")
    bnsc_e = nc.dram_tensor("bnsc", [P, OB], f32, kind="ExternalInput")
    bnbi_e = nc.dram_tensor("bnbi", [P, OB], f32, kind="ExternalInput")
    # outT: [OUTD, ROWS]; the host transposes each core's block.
    out_e = nc.dram_tensor("out", [OUTD, ROWS], f32, kind="ExternalOutput")

    groups = [list(range(NCORES))]

    def allgather(g_in, g_out):
        nc.gpsimd.collective_compute(
            "AllGather", mybir.AluOpType.bypass, replica_groups=groups,
            ins=[g_in[:]], outs=[g_out[:]])

    with tile.TileContext(nc) as tc:
        with (
            tc.tile_pool(name="const", bufs=1) as const,
            tc.tile_pool(name="xslab", bufs=4) as xslab_p,
            tc.tile_pool(name="sloc", bufs=1) as sloc_p,
            tc.tile_pool(name="sred", bufs=1) as sred_p,
            tc.tile_pool(name="sgt", bufs=(NCORES - NRED) * IB) as sgt_p,
            tc.tile_pool(name="h1", bufs=1) as h1_p,
            tc.tile_pool(name="zsb", bufs=1) as z_p,
            tc.tile_pool(name="zchunk", bufs=8) as zchunk_p,
            tc.tile_pool(name="astream", bufs=24) as astream,
            tc.tile_pool(name="apair", bufs=12) as apair_p,
            tc.tile_pool(name="outsb", bufs=1) as outsb_p,
            tc.tile_pool(name="dram", bufs=1, space="DRAM") as dram,
        ):
            # ---- phase-A constants as slab DMAs (first matmul starts
            # once w1 slab 0 + xT slab 0 land)
            w1_sb = const.tile([P, CB, NHID], bf16)
            for cb in range(CB):
                nc.sync.dma_start(w1_sb[:, cb, :],
                                  w1_e[cb * P:(cb + 1) * P, :])

            # ---- DRAM bounce buffers for the collectives
            RPC = ROWS // GC  # rows bounced per S chunk (512)
            s_bounce = [dram.tile([RPC, NHID], bf16, name=f"sb{c}")
                        for c in range(GC)]
            # s_all[c] row k*RPC + r = S[global row k*ROWS + c*RPC + r]
            s_all = [dram.tile([RPC * NCORES, NHID], bf16,
                               addr_space="Shared", name=f"sa{c}")
                     for c in range(GC)]
            # z bounce is p-major: row p holds (t, o), t = chunk-local
            # i-block -> gathered z is plain-sliceable per core block
            z_bounce = [dram.tile([P, QT * OUTD], f8, name=f"zb{c}")
                        for c in range(GC)]
            z_all = [dram.tile([P * NCORES, QT * OUTD], f8,
                               addr_space="Shared", name=f"za{c}")
                     for c in range(GC)]

            s_loc = sloc_p.tile([P, IB, NHID], bf16)
            s_red = sred_p.tile([P, NRED * IB, NHID], bf16)

            # ---- phase A: own S block, cb-outer across 8 PSUM banks;
            # bounce per half, fire the two S AllGather chunks early.
            with tc.tile_pool(name="psA", bufs=1, space="PSUM") as psA:
                ps_own = [psA.tile([P, NHID], f32, name=f"pso{ib}",
                                   tag=f"pa{ib}")
                          for ib in range(IB)]
                for cb in range(CB):
                    xs = xslab_p.tile([P, ROWS], bf16, tag="xslab")
                    nc.sync.dma_start(xs[:], xT_e[cb * P:(cb + 1) * P, :])
                    for ib in range(IB):
                        nc.tensor.matmul(
                            ps_own[ib][:], xs[:, ib * P:(ib + 1) * P],
                            w1_sb[:, cb, :],
                            start=(cb == 0), stop=(cb == CB - 1))
                for c in range(GC):
                    for t in range(IB // GC):
                        ib = c * (IB // GC) + t
                        nc.scalar.activation(s_loc[:, ib, :],
                                             ps_own[ib][:], AF.Copy)
                        nc.sync.dma_start(
                            s_bounce[c][t * P:(t + 1) * P, :],
                            s_loc[:, ib, :])
                    allgather(s_bounce[c], s_all[c])

                # remaining constants (needed from phase C on)
                b1p_sb = const.tile([P, JB], f32)
                nc.sync.dma_start(b1p_sb[:], b1p_e[:])
                w2_sb = const.tile([P, JB, OUTD], bf16)
                nc.sync.dma_start(w2_sb[:], w2_e[:])
                bnsc_sb = const.tile([P, OB], f32)
                nc.sync.dma_start(bnsc_sb[:], bnsc_e[:])
                bnbi_sb = const.tile([P, OB], f32)
                nc.sync.dma_start(bnbi_sb[:], bnbi_e[:])

                # ---- phase B: redundant S for global blocks 5..7
                for r in range(NRED):
                    ps_r = [psA.tile([P, NHID], f32, name=f"psr{r}_{ib}",
                                     tag=f"pa{ib}")
                            for ib in range(IB)]
                    for cb in range(CB):
                        xs = xslab_p.tile([P, ROWS], bf16, tag="xslab")
                        nc.sync.dma_start(
                            xs[:],
                            xTr_e[cb * P:(cb + 1) * P,
                                  r * ROWS:(r + 1) * ROWS])
                        for ib in range(IB):
                            nc.tensor.matmul(
                                ps_r[ib][:], xs[:, ib * P:(ib + 1) * P],
                                w1_sb[:, cb, :],
                                start=(cb == 0), stop=(cb == CB - 1))
                    for ib in range(IB):
                        nc.scalar.activation(
                            s_red[:, r * IB + ib, :], ps_r[ib][:], AF.Copy)

            h1T = h1_p.tile([P, JB, ROWS], bf16)
            z_sb = z_p.tile([P, IB, OUTD], f8)
            s_gt = {}

            # m-traversal per pass: local blocks 5..7, then gathered
            # blocks chunk-major (all chunk-0 quarters before chunk-1).
            walk = ([(g, q) for g in range(RED0, NCORES)
                     for q in range(IB)]
                    + [(g, c * QT + t) for c in range(GC)
                       for g in range(RED0) for t in range(QT)])

            # ---- phase C, i-half pass ih. Streaming a-tiles go on the
            # Activation HW-DGE queue; gathered-S staging on the SP one.
            def l1_pass(ih, psh, psz):
                psum_h = [psh.tile([P, HF], f32, name=f"ph{jb}_{ih}",
                                   tag=f"ph{jb}")
                          for jb in range(JB)]
                for n_emitted, (g, q) in enumerate(walk):
                    if g >= RED0:
                        s_src = s_red[:, (g - RED0) * IB + q, :]
                    else:
                        if ih == 0 and s_gt.get((g, q)) is None:
                            c, t = divmod(q, QT)
                            st = sgt_p.tile([P, NHID], bf16,
                                            name=f"sg{g}_{q}", tag="sgt")
                            nc.sync.dma_start(
                                st[:],
                                s_all[c][(g * QT + t) * P:
                                         (g * QT + t + 1) * P, :])
                            s_gt[(g, q)] = st
                        s_src = s_gt[(g, q)][:]
                    mt = g * IB + q
                    a_tile = astream.tile([P, HF], f8, tag="ahalf")
                    nc.scalar.dma_start(
                        a_tile[:],
                        ifadjT_e[mt * P:(mt + 1) * P,
                                 ih * HF:(ih + 1) * HF])
                    for jb in range(JB):
                        nc.tensor.matmul(
                            psum_h[jb][:],
                            s_src[:, jb * P:(jb + 1) * P],
                            a_tile[:],
                            start=(n_emitted == 0),
                            stop=(n_emitted == MT - 1),
                        )
                # epilogue: relu + (b1 + colsum/2) bias into h1T half
                for jb in range(JB):
                    nc.scalar.activation(
                        h1T[:, jb, ih * HF:(ih + 1) * HF],
                        psum_h[jb][:], AF.Relu,
                        bias=b1p_sb[:, jb:jb + 1])
                # z for this half's i-blocks (fp8, W2 pre-scaled by 1/4),
                # p-major bounce, gather chunk ih
                for t in range(IB // IH):
                    ib = ih * (IB // IH) + t
                    ps = psz.tile([P, OUTD], f32, tag="z")
                    for jb in range(JB):
                        nc.tensor.matmul(
                            ps[:],
                            h1T[:, jb, ib * P:(ib + 1) * P],
                            w2_sb[:, jb, :],
                            start=(jb == 0), stop=(jb == JB - 1),
                        )
                    nc.scalar.activation(z_sb[:, ib, :], ps[:], AF.Copy)
                    nc.sync.dma_start(
                        z_bounce[ih][:, t * OUTD:(t + 1) * OUTD],
                        z_sb[:, ib, :])
                allgather(z_bounce[ih], z_all[ih])

            with (
                tc.tile_pool(name="psh", bufs=1, space="PSUM") as psh,
                tc.tile_pool(name="psz", bufs=2, space="PSUM") as psz,
            ):
                for ih in range(IH):
                    l1_pass(ih, psh, psz)

            # ---- phase D: outT[o, i] = sum_m Z[m, o] * adjT[m, i]
            # fp8 DoubleRow, one matmul per adjacent m-tile pair.
            # z_all[c] row k*P+p holds (t, o) = z[k*ROWS + c*RPC + t*P+p]
            # -> m-tile of (c, k, t) is 8k + 4c + t.
            outT_sb = outsb_p.tile([P, OB, ROWS], f32)
            with tc.tile_pool(name="ps4", bufs=1, space="PSUM") as ps4:
                psum_o = [[ps4.tile([P, HF], f32, name=f"po{ob}_{ih}",
                                    tag=f"po{ob}_{ih}")
                           for ih in range(IH)] for ob in range(OB)]
                first = True
                for c in range(GC):
                    for k in range(NCORES):
                        zc_sb = zchunk_p.tile([P, QT, OUTD], f8,
                                              tag="zchunk")
                        nc.sync.dma_start(
                            zc_sb[:], z_all[c][k * P:(k + 1) * P, :])
                        for pr in range(0, QT, 2):
                            mt = IB * k + QT * c + pr
                            a_pair = apair_p.tile([P, 2, ROWS], f8,
                                                  tag="apair")
                            nc.scalar.dma_start(
                                a_pair[:],
                                adjP_e[(mt // 2) * P:(mt // 2 + 1) * P, :])
                            last_pr = (c == GC - 1 and k == NCORES - 1
                                       and pr == QT - 2)
                            for ob in range(OB):
                                for ih in range(IH):
                                    nc.tensor.matmul(
                                        psum_o[ob][ih][:],
                                        zc_sb[:, pr:pr + 2,
                                              ob * P:(ob + 1) * P],
                                        a_pair[:, :,
                                               ih * HF:(ih + 1) * HF],
                                        start=first, stop=last_pr,
                                        perf_mode=DR,
                                    )
                            first = False
                # fused BN affine on PSUM evict: out = psum*scale + bias
                for ob in range(OB):
                    for ih in range(IH):
                        nc.vector.tensor_scalar(
                            outT_sb[:, ob, ih * HF:(ih + 1) * HF],
                            psum_o[ob][ih][:],
                            bnsc_sb[:, ob:ob + 1],
                            bnbi_sb[:, ob:ob + 1],
                            mybir.AluOpType.mult,
                            mybir.AluOpType.add)
                    nc.sync.dma_start(
                        out_e[ob * P:(ob + 1) * P, :], outT_sb[:, ob, :])

    nc.compile()
    return nc


def _get_nc():
    if "nc" not in _cache:
        _cache["nc"] = _build()
    return _cache["nc"]


def kernel(x, IFadj, adj, W1, b1, W2, b2, bn_gamma, bn_beta, bn_mean, bn_var):
    from concourse.bass_utils import run_bass_kernel_spmd

    x = np.asarray(x, dtype=np.float32)
    IFadj = np.asarray(IFadj, dtype=np.float32)
    adj = np.asarray(adj, dtype=np.float32)
    W1 = np.asarray(W1, dtype=np.float32)
    b1 = np.asarray(b1, dtype=np.float32)
    W2 = np.asarray(W2, dtype=np.float32)
    b2 = np.asarray(b2, dtype=np.float32)
    bn_gamma = np.asarray(bn_gamma, dtype=np.float32)
    bn_beta = np.asarray(bn_beta, dtype=np.float32)
    bn_mean = np.asarray(bn_mean, dtype=np.float32)
    bn_var = np.asarray(bn_var, dtype=np.float32)

    # host-side prep: shard rows, transpose for PE lhsT layout, cast.
    # W2 is pre-scaled by 1/4 so z stays well inside fp8e4 range; the
    # BN scale is multiplied by 4 to undo it after the layer-2 spmm.
    w1b = W1.astype(_BF16)
    w2b = np.ascontiguousarray(
        (W2 * 0.25).astype(_BF16).reshape(JB, P, OUTD)
        .transpose(1, 0, 2).reshape(P, JB * OUTD))
    # layer-1 bias including the exact 1/2*colsum(S) centering term
    colsum = x.sum(axis=0, dtype=np.float64).astype(np.float32) @ W1
    b1c = b1 + 0.5 * colsum
    b1p = np.ascontiguousarray(b1c.reshape(JB, P).T)  # [P, JB]
    inv = bn_gamma / np.sqrt(bn_var + BN_EPS)
    bias_tot = b2 * inv + bn_beta - bn_mean * inv
    bnsc = np.ascontiguousarray((4.0 * inv).reshape(OB, P).T)   # [P, OB]
    bnbi = np.ascontiguousarray(bias_tot.reshape(OB, P).T)      # [P, OB]

    # replicated x rows for global node blocks 5..7
    xTr = np.ascontiguousarray(x[RED0 * ROWS:].T).astype(_BF16)

    in_maps = []
    for k in range(NCORES):
        r0, r1 = k * ROWS, (k + 1) * ROWS
        adjT8 = np.ascontiguousarray(adj[r0:r1].T).astype(_F8)  # [N, ROWS]
        # pair-interleave: row pair*P+p = m-tiles (2p, 2p+1) side by side
        adjP = np.ascontiguousarray(
            adjT8.reshape(N // 256, 2, P, ROWS).transpose(0, 2, 1, 3)
            .reshape(N // 2, 2 * ROWS))
        in_maps.append({
            "xT": np.ascontiguousarray(x[r0:r1].T).astype(_BF16),
            "xTr": xTr,
            "ifadjT": np.ascontiguousarray(
                IFadj[r0:r1].T - np.float32(0.5)).astype(_F8),
            "adjP": adjP,
            "w1": w1b,
            "w2": w2b,
            "b1p": b1p,
            "bnsc": bnsc,
            "bnbi": bnbi,
        })

    global _last_in_maps
    _last_in_maps = in_maps

    nc = _get_nc()
    try:
        res = run_bass_kernel_spmd(nc, in_maps, list(range(NCORES)))
    except Exception:
        # transient device wedge (NRT_EXEC_UNIT_UNRECOVERABLE etc.) --
        # a straight retry has been observed to recover
        import time
        time.sleep(2.0)
        res = run_bass_kernel_spmd(nc, in_maps, list(range(NCORES)))
    # per-core output is outT [OUTD, ROWS]; transpose back and stack rows
    return np.concatenate(
        [np.ascontiguousarray(res.results[k]["out"].T)
         for k in range(NCORES)], axis=0)


# revision 11
# speedup vs baseline: 1.1306x; 1.0732x over previous
"""Trainium2 Bass kernel for a 2-layer DGCN (graph conv) on 8 NeuronCores.

Reference computation (fp32):
    h1  = relu(IFadj @ (x @ W1) + b1)         # [N, NHID]
    out = BN(adj @ (h1 @ W2) + b2)            # [N, OUTD], BN in eval mode

Distribution: rows of x / IFadj / adj are sharded across 8 cores
(row-parallel graph partitioning). Per core (rows R_k), v6 schedule:

  phase A: S_own = x[R_k] @ W1 (cb-outer over 8 PSUM banks, x/W1
           streamed in interleaved 128-row slabs); two pipelined
           S-AllGather chunks fired as soon as their bounce lands,
           with the triggers pinned to scheduler priority 0. The
           first collective absorbs the cross-core launch-skew
           barrier (~45us, unavoidable) while local compute runs.
  phase B: redundantly compute S for global node blocks 5..7 (same on
           every core, from a replicated x slice): local work that
           covers the barrier + gather window.
  phase C: h1T = relu(S^T @ B + bias), two i-half passes; within each
           pass, local m-blocks 5..7 first (no collective dep), then
           gathered blocks 0..4 chunk-major. After each pass: z-half
           = h1 @ (W2/4) in fp8, Z-AllGather chunk fired mid-kernel.
  phase D: outT = Z-as-lhsT vs adjT_k rhs, fp8e4 DoubleRow matmuls
           (one instruction per adjacent m-tile pair); BN fused on
           the PSUM evict with the x4 range-fold undone in the BN
           scale.

Engine/queue discipline (the v2-v5 lessons): every dma_start
dispatches serially on its issuing engine's queue, and the tile
scheduler may reorder same-engine dispatches, so a collective-gated
DMA can head-block urgent loads behind it. Therefore:
  - SP (nc.sync) queue: compute-paced setup only (w1/x/xTr slabs,
    bounce writes, consts, out writes).
  - Activation (nc.scalar) queue: the big streaming loads (IFadj
    dual-m-tile tiles, adj pair tiles) -- never gated on collectives.
  - GpSimd software DGE: ONLY the collective-gated staging (gathered
    S quarters, gathered z blocks); a head-block there delays nothing
    else. The S-AllGather triggers (also gpsimd) carry priority 0 so
    staging can never be scheduled ahead of them.
  - Plain-Copy PSUM evictions run on the Vector engine, keeping the
    Activation engine free for relu evictions + its DMA queue.
All transfers are plain 2D/3D slices: the host pre-permutes IFadj and
adj into pair-interleaved layouts, W2 partition-major, and the z
bounce is written p-major so gathered z is plain-sliceable.

Precision: layer-1 runs lhsT=S in bf16 against the IFadj moving
operand in fp8e4, CENTERED: B = IFadj - 1/2 (entries U[0,1] ->
[-1/2,1/2]); the exact mean term 1/2*colsum(S)_j is folded into the
layer-1 bias on the host (colsum(S) = (sum_m x[m,:]) @ W1, a trivial
host matvec). Measured end-to-end rel err ~2.2e-3 vs the 2e-2 gate.
The layer-2 spmm runs both operands fp8e4 with z pre-scaled by 1/4 to
sit far inside e4m3 range.
"""

import numpy as np
import ml_dtypes

NCORES = 8
N = 8192
NFEAT = 1024
NHID = 512
OUTD = 256
ROWS = N // NCORES  # 1024
P = 128
BN_EPS = 1e-5

CB = NFEAT // P   # 8  c-blocks (x feature contraction)
IB = ROWS // P    # 8  i-blocks per node block
JB = NHID // P    # 4  j-blocks (hidden)
MT = N // P       # 64 m-tiles (global node contraction)
HF = 512          # matmul moving free dim (PSUM bank limit)
IH = ROWS // HF   # 2 i-halves of the local row range
OB = OUTD // P    # 2 output-feature blocks
GC = 2            # allgather chunks (S and Z)
QT = 4            # m-tiles per (chunk, block) quarter
NRED = 3          # redundant S blocks (global blocks 5..7)
RED0 = NCORES - NRED  # first redundant block = 5

_BF16 = ml_dtypes.bfloat16
_F8 = ml_dtypes.float8_e4m3

_cache = {}


def _build():
    import concourse.mybir as mybir
    import concourse.tile as tile
    from concourse import bacc

    dt = mybir.dt
    f32 = dt.float32
    bf16 = dt.bfloat16
    f8 = dt.float8e4
    AF = mybir.ActivationFunctionType
    DR = mybir.MatmulPerfMode.DoubleRow
    MULT = mybir.AluOpType.mult

    nc = bacc.Bacc("TRN2", target_bir_lowering=False, debug=False,
                   num_devices=NCORES)

    xT_e = nc.dram_tensor("xT", [NFEAT, ROWS], bf16, kind="ExternalInput")
    # replicated x rows for global node blocks 5..7 (same on every core)
    xTr_e = nc.dram_tensor("xTr", [NFEAT, NRED * ROWS], bf16,
                           kind="ExternalInput")
    # centered IFadj^T in fp8, pair-interleaved and split by column
    # half on the host: row ih*4096 + pair*P + p, col t*HF + c holds
    # IFadjT[(2*pair+t)*P + p, ih*HF + c] - 1/2
    ifadjH_e = nc.dram_tensor("ifadjH", [IH * N // 2, 2 * HF], f8,
                              kind="ExternalInput")
    # adj rows pair-interleaved on host: row pair*P+p holds m-tiles
    # (2*pair, 2*pair+1) side by side -> [P, 2, ROWS] is a plain slice
    adjP_e = nc.dram_tensor("adjP", [N // 2, 2 * ROWS], f8,
                            kind="ExternalInput")
    w1_e = nc.dram_tensor("w1", [NFEAT, NHID], bf16, kind="ExternalInput")
    # W2/4, partition-major: [P, JB*OUTD]
    w2_e = nc.dram_tensor("w2", [P, JB * OUTD], bf16, kind="ExternalInput")
    # layer-1 bias + 1/2*colsum(S) fold, [P, JB]
    b1p_e = nc.dram_tensor("b1p", [P, JB], f32, kind="ExternalInput")
    bnsc_e = nc.dram_tensor("bnsc", [P, OB], f32, kind="ExternalInput")
    bnbi_e = nc.dram_tensor("bnbi", [P, OB], f32, kind="ExternalInput")
    # outT: [OUTD, ROWS]; the host transposes each core's block.
    out_e = nc.dram_tensor("out", [OUTD, ROWS], f32, kind="ExternalOutput")

    groups = [list(range(NCORES))]

    def allgather(g_in, g_out):
        nc.gpsimd.collective_compute(
            "AllGather", mybir.AluOpType.bypass, replica_groups=groups,
            ins=[g_in[:]], outs=[g_out[:]])

    with tile.TileContext(nc) as tc:
        with (
            tc.tile_pool(name="const", bufs=1) as const,
            tc.tile_pool(name="xslab", bufs=4) as xslab_p,
            tc.tile_pool(name="sloc", bufs=1) as sloc_p,
            tc.tile_pool(name="sred", bufs=1) as sred_p,
            tc.tile_pool(name="sgt", bufs=(NCORES - NRED) * IB) as sgt_p,
            tc.tile_pool(name="h1", bufs=1) as h1_p,
            tc.tile_pool(name="zsb", bufs=1) as z_p,
            tc.tile_pool(name="zchunk", bufs=8) as zchunk_p,
            tc.tile_pool(name="astream", bufs=12) as astream,
            tc.tile_pool(name="apair", bufs=12) as apair_p,
            tc.tile_pool(name="outsb", bufs=1) as outsb_p,
            tc.tile_pool(name="dram", bufs=1, space="DRAM") as dram,
        ):
            # ---- DRAM bounce buffers for the collectives
            RPC = ROWS // GC  # rows bounced per S chunk (512)
            s_bounce = [dram.tile([RPC, NHID], bf16, name=f"sb{c}")
                        for c in range(GC)]
            # s_all[c] row k*RPC + r = S[global row k*ROWS + c*RPC + r]
            s_all = [dram.tile([RPC * NCORES, NHID], bf16,
                               addr_space="Shared", name=f"sa{c}")
                     for c in range(GC)]
            # z bounce is p-major: row p holds (t, o), t = chunk-local
            # i-block -> gathered z is plain-sliceable per core block
            z_bounce = [dram.tile([P, QT * OUTD], f8, name=f"zb{c}")
                        for c in range(GC)]
            z_all = [dram.tile([P * NCORES, QT * OUTD], f8,
                               addr_space="Shared", name=f"za{c}")
                     for c in range(GC)]

            s_loc = sloc_p.tile([P, IB, NHID], bf16)
            s_red = sred_p.tile([P, NRED * IB, NHID], bf16)
            w1_sb = const.tile([P, CB, NHID], bf16)

            # ---- phase A: own S block, cb-outer across 8 PSUM banks;
            # x/W1 slab loads interleaved so the first matmul starts as
            # soon as (w1 slab 0, x slab 0) land. Bounce per half, fire
            # the two S AllGather chunks at top scheduler priority.
            with tc.tile_pool(name="psA", bufs=1, space="PSUM") as psA:
                ps_own = [psA.tile([P, NHID], f32, name=f"pso{ib}",
                                   tag=f"pa{ib}")
                          for ib in range(IB)]
                for cb in range(CB):
                    nc.sync.dma_start(w1_sb[:, cb, :],
                                      w1_e[cb * P:(cb + 1) * P, :])
                    xs = xslab_p.tile([P, ROWS], bf16, tag="xslab")
                    nc.sync.dma_start(xs[:], xT_e[cb * P:(cb + 1) * P, :])
                    for ib in range(IB):
                        nc.tensor.matmul(
                            ps_own[ib][:], xs[:, ib * P:(ib + 1) * P],
                            w1_sb[:, cb, :],
                            start=(cb == 0), stop=(cb == CB - 1))
                for c in range(GC):
                    for t in range(IB // GC):
                        ib = c * (IB // GC) + t
                        nc.vector.tensor_scalar(
                            s_loc[:, ib, :], ps_own[ib][:], 1.0, None,
                            MULT)
                        nc.sync.dma_start(
                            s_bounce[c][t * P:(t + 1) * P, :],
                            s_loc[:, ib, :])
                    with tc.high_priority():
                        allgather(s_bounce[c], s_all[c])

                # remaining constants (needed from phase C on)
                b1p_sb = const.tile([P, JB], f32)
                nc.sync.dma_start(b1p_sb[:], b1p_e[:])
                w2_sb = const.tile([P, JB, OUTD], bf16)
                nc.sync.dma_start(w2_sb[:], w2_e[:])
                bnsc_sb = const.tile([P, OB], f32)
                nc.sync.dma_start(bnsc_sb[:], bnsc_e[:])
                bnbi_sb = const.tile([P, OB], f32)
                nc.sync.dma_start(bnbi_sb[:], bnbi_e[:])

                # ---- phase B: redundant S for global blocks 5..7
                for r in range(NRED):
                    ps_r = [psA.tile([P, NHID], f32, name=f"psr{r}_{ib}",
                                     tag=f"pa{ib}")
                            for ib in range(IB)]
                    for cb in range(CB):
                        xs = xslab_p.tile([P, ROWS], bf16, tag="xslab")
                        nc.sync.dma_start(
                            xs[:],
                            xTr_e[cb * P:(cb + 1) * P,
                                  r * ROWS:(r + 1) * ROWS])
                        for ib in range(IB):
                            nc.tensor.matmul(
                                ps_r[ib][:], xs[:, ib * P:(ib + 1) * P],
                                w1_sb[:, cb, :],
                                start=(cb == 0), stop=(cb == CB - 1))
                    for ib in range(IB):
                        nc.vector.tensor_scalar(
                            s_red[:, r * IB + ib, :], ps_r[ib][:], 1.0,
                            None, MULT)

            h1T = h1_p.tile([P, JB, ROWS], bf16)
            z_sb = z_p.tile([P, IB, OUTD], f8)
            s_gt = {}

            # m-traversal per pass, in adjacent PAIRS (one IFadj dual
            # tile feeds two m-tiles): local blocks 5..7 first, then
            # gathered blocks chunk-major.
            pair_walk = ([(g, q) for g in range(RED0, NCORES)
                          for q in range(0, IB, 2)]
                         + [(g, c * QT + t) for c in range(GC)
                            for g in range(RED0)
                            for t in range(0, QT, 2)])

            # ---- phase C, i-half pass ih. Streaming dual tiles on the
            # Activation queue; gathered-S staging on the gpsimd DGE.
            def l1_pass(ih, psh, psz):
                psum_h = [psh.tile([P, HF], f32, name=f"ph{jb}_{ih}",
                                   tag=f"ph{jb}")
                          for jb in range(JB)]
                n_emitted = 0
                for g, q in pair_walk:
                    a_dual = astream.tile([P, 2, HF], f8, tag="adual")
                    pidx = (g * IB + q) // 2
                    nc.scalar.dma_start(
                        a_dual[:],
                        ifadjH_e[ih * (N // 2) + pidx * P:
                                 ih * (N // 2) + (pidx + 1) * P, :])
                    for u in range(2):
                        qq = q + u
                        if g >= RED0:
                            s_src = s_red[:, (g - RED0) * IB + qq, :]
                        else:
                            if ih == 0 and s_gt.get((g, qq)) is None:
                                c, t = divmod(qq, QT)
                                st = sgt_p.tile([P, NHID], bf16,
                                                name=f"sg{g}_{qq}",
                                                tag="sgt")
                                nc.gpsimd.dma_start(
                                    st[:],
                                    s_all[c][(g * QT + t) * P:
                                             (g * QT + t + 1) * P, :])
                                s_gt[(g, qq)] = st
                            s_src = s_gt[(g, qq)][:]
                        for jb in range(JB):
                            nc.tensor.matmul(
                                psum_h[jb][:],
                                s_src[:, jb * P:(jb + 1) * P],
                                a_dual[:, u, :],
                                start=(n_emitted == 0),
                                stop=(n_emitted == MT - 1),
                            )
                        n_emitted += 1
                # epilogue: relu + (b1 + colsum/2) bias into h1T half
                for jb in range(JB):
                    nc.scalar.activation(
                        h1T[:, jb, ih * HF:(ih + 1) * HF],
                        psum_h[jb][:], AF.Relu,
                        bias=b1p_sb[:, jb:jb + 1])
                # z for this half's i-blocks (fp8, W2 pre-scaled by 1/4),
                # p-major bounce, gather chunk ih
                for t in range(IB // IH):
                    ib = ih * (IB // IH) + t
                    ps = psz.tile([P, OUTD], f32, tag="z")
                    for jb in range(JB):
                        nc.tensor.matmul(
                            ps[:],
                            h1T[:, jb, ib * P:(ib + 1) * P],
                            w2_sb[:, jb, :],
                            start=(jb == 0), stop=(jb == JB - 1),
                        )
                    nc.vector.tensor_scalar(
                        z_sb[:, ib, :], ps[:], 1.0, None, MULT)
                    nc.sync.dma_start(
                        z_bounce[ih][:, t * OUTD:(t + 1) * OUTD],
                        z_sb[:, ib, :])
                allgather(z_bounce[ih], z_all[ih])

            with (
                tc.tile_pool(name="psh", bufs=1, space="PSUM") as psh,
                tc.tile_pool(name="psz", bufs=2, space="PSUM") as psz,
            ):
                for ih in range(IH):
                    l1_pass(ih, psh, psz)

            # ---- phase D: outT[o, i] = sum_m Z[m, o] * adjT[m, i]
            # fp8 DoubleRow, one matmul per adjacent m-tile pair.
            # z_all[c] row k*P+p holds (t, o) = z[k*ROWS + c*RPC + t*P+p]
            # -> m-tile of (c, k, t) is 8k + 4c + t.
            outT_sb = outsb_p.tile([P, OB, ROWS], f32)
            with tc.tile_pool(name="ps4", bufs=1, space="PSUM") as ps4:
                psum_o = [[ps4.tile([P, HF], f32, name=f"po{ob}_{ih}",
                                    tag=f"po{ob}_{ih}")
                           for ih in range(IH)] for ob in range(OB)]
                first = True
                for c in range(GC):
                    for k in range(NCORES):
                        zc_sb = zchunk_p.tile([P, QT, OUTD], f8,
                                              tag="zchunk")
                        nc.gpsimd.dma_start(
                            zc_sb[:], z_all[c][k * P:(k + 1) * P, :])
                        for pr in range(0, QT, 2):
                            mt = IB * k + QT * c + pr
                            a_pair = apair_p.tile([P, 2, ROWS], f8,
                                                  tag="apair")
                            nc.scalar.dma_start(
                                a_pair[:],
                                adjP_e[(mt // 2) * P:(mt // 2 + 1) * P, :])
                            last_pr = (c == GC - 1 and k == NCORES - 1
                                       and pr == QT - 2)
                            for ob in range(OB):
                                for ih in range(IH):
                                    nc.tensor.matmul(
                                        psum_o[ob][ih][:],
                                        zc_sb[:, pr:pr + 2,
                                              ob * P:(ob + 1) * P],
                                        a_pair[:, :,
                                               ih * HF:(ih + 1) * HF],
                                        start=first, stop=last_pr,
                                        perf_mode=DR,
                                    )
                            first = False
                # fused BN affine on PSUM evict: out = psum*scale + bias
                for ob in range(OB):
                    for ih in range(IH):
                        nc.vector.tensor_scalar(
                            outT_sb[:, ob, ih * HF:(ih + 1) * HF],
                            psum_o[ob][ih][:],
                            bnsc_sb[:, ob:ob + 1],
                            bnbi_sb[:, ob:ob + 1],
                            mybir.AluOpType.mult,
                            mybir.AluOpType.add)
                    nc.sync.dma_start(
                        out_e[ob * P:(ob + 1) * P, :], outT_sb[:, ob, :])

    nc.compile()
    return nc


def _get_nc():
    if "nc" not in _cache:
        _cache["nc"] = _build()
    return _cache["nc"]


def kernel(x, IFadj, adj, W1, b1, W2, b2, bn_gamma, bn_beta, bn_mean, bn_var):
    from concourse.bass_utils import run_bass_kernel_spmd

    x = np.asarray(x, dtype=np.float32)
    IFadj = np.asarray(IFadj, dtype=np.float32)
    adj = np.asarray(adj, dtype=np.float32)
    W1 = np.asarray(W1, dtype=np.float32)
    b1 = np.asarray(b1, dtype=np.float32)
    W2 = np.asarray(W2, dtype=np.float32)
    b2 = np.asarray(b2, dtype=np.float32)
    bn_gamma = np.asarray(bn_gamma, dtype=np.float32)
    bn_beta = np.asarray(bn_beta, dtype=np.float32)
    bn_mean = np.asarray(bn_mean, dtype=np.float32)
    bn_var = np.asarray(bn_var, dtype=np.float32)

    # host-side prep: shard rows, transpose for PE lhsT layout, cast.
    # W2 is pre-scaled by 1/4 so z stays well inside fp8e4 range; the
    # BN scale is multiplied by 4 to undo it after the layer-2 spmm.
    w1b = W1.astype(_BF16)
    w2b = np.ascontiguousarray(
        (W2 * 0.25).astype(_BF16).reshape(JB, P, OUTD)
        .transpose(1, 0, 2).reshape(P, JB * OUTD))
    # layer-1 bias including the exact 1/2*colsum(S) centering term
    colsum = x.sum(axis=0, dtype=np.float64).astype(np.float32) @ W1
    b1c = b1 + 0.5 * colsum
    b1p = np.ascontiguousarray(b1c.reshape(JB, P).T)  # [P, JB]
    inv = bn_gamma / np.sqrt(bn_var + BN_EPS)
    bias_tot = b2 * inv + bn_beta - bn_mean * inv
    bnsc = np.ascontiguousarray((4.0 * inv).reshape(OB, P).T)   # [P, OB]
    bnbi = np.ascontiguousarray(bias_tot.reshape(OB, P).T)      # [P, OB]

    # replicated x rows for global node blocks 5..7
    xTr = np.ascontiguousarray(x[RED0 * ROWS:].T).astype(_BF16)

    in_maps = []
    for k in range(NCORES):
        r0, r1 = k * ROWS, (k + 1) * ROWS
        # centered IFadj^T in fp8: [m, col] -> [ih, pair, p, t, c]
        A8 = (IFadj[r0:r1].T - np.float32(0.5)).astype(_F8)  # [N, ROWS]
        ifadjH = np.ascontiguousarray(
            A8.reshape(N // 256, 2, P, IH, HF).transpose(3, 0, 2, 1, 4)
            .reshape(IH * N // 2, 2 * HF))
        adjT8 = np.ascontiguousarray(adj[r0:r1].T).astype(_F8)  # [N, ROWS]
        # pair-interleave: row pair*P+p = m-tiles (2p, 2p+1) side by side
        adjP = np.ascontiguousarray(
            adjT8.reshape(N // 256, 2, P, ROWS).transpose(0, 2, 1, 3)
            .reshape(N // 2, 2 * ROWS))
        in_maps.append({
            "xT": np.ascontiguousarray(x[r0:r1].T).astype(_BF16),
            "xTr": xTr,
            "ifadjH": ifadjH,
            "adjP": adjP,
            "w1": w1b,
            "w2": w2b,
            "b1p": b1p,
            "bnsc": bnsc,
            "bnbi": bnbi,
        })

    global _last_in_maps
    _last_in_maps = in_maps

    nc = _get_nc()
    try:
        res = run_bass_kernel_spmd(nc, in_maps, list(range(NCORES)))
    except Exception:
        # transient device wedge (NRT_EXEC_UNIT_UNRECOVERABLE etc.) --
        # a straight retry has been observed to recover
        import time
        time.sleep(2.0)
        res = run_bass_kernel_spmd(nc, in_maps, list(range(NCORES)))
    # per-core output is outT [OUTD, ROWS]; transpose back and stack rows
    return np.concatenate(
        [np.ascontiguousarray(res.results[k]["out"].T)
         for k in range(NCORES)], axis=0)


# revision 14
# speedup vs baseline: 1.2254x; 1.0839x over previous
"""Trainium2 Bass kernel for a 2-layer DGCN (graph conv) on 8 NeuronCores.

Reference computation (fp32):
    h1  = relu(IFadj @ (x @ W1) + b1)         # [N, NHID]
    out = BN(adj @ (h1 @ W2) + b2)            # [N, OUTD], BN in eval mode

Distribution: rows of x / IFadj / adj are sharded across 8 cores
(row-parallel graph partitioning). Per core (rows R_k), v6 schedule:

  phase A: S_own = x[R_k] @ W1 (cb-outer over 8 PSUM banks, x/W1
           streamed in interleaved 128-row slabs); two pipelined
           S-AllGather chunks fired as soon as their bounce lands,
           with the triggers pinned to scheduler priority 0. The
           first collective absorbs the cross-core launch-skew
           barrier (~45us, unavoidable) while local compute runs.
  phase B: redundantly compute S for global node blocks 5..7 (same on
           every core, from a replicated x slice): local work that
           covers the barrier + gather window.
  phase C: h1T = relu(S^T @ B + bias), two i-half passes; within each
           pass, local m-blocks 5..7 first (no collective dep), then
           gathered blocks 0..4 chunk-major. After each pass: z-half
           = h1 @ (W2/4) in fp8, Z-AllGather chunk fired mid-kernel.
  phase D: outT = Z-as-lhsT vs adjT_k rhs, fp8e4 DoubleRow matmuls
           (one instruction per adjacent m-tile pair); BN fused on
           the PSUM evict with the x4 range-fold undone in the BN
           scale.

Engine/queue discipline (the v2-v5 lessons): every dma_start
dispatches serially on its issuing engine's queue, and the tile
scheduler may reorder same-engine dispatches, so a collective-gated
DMA can head-block urgent loads behind it. Therefore:
  - SP (nc.sync) queue: compute-paced setup only (w1/x/xTr slabs,
    bounce writes, consts, out writes).
  - Activation (nc.scalar) queue: the big streaming loads (IFadj
    dual-m-tile tiles, adj pair tiles) -- never gated on collectives.
  - GpSimd software DGE: ONLY the collective-gated staging (gathered
    S quarters, gathered z blocks); a head-block there delays nothing
    else. The S-AllGather triggers (also gpsimd) carry priority 0 so
    staging can never be scheduled ahead of them.
  - Plain-Copy PSUM evictions run on the Vector engine, keeping the
    Activation engine free for relu evictions + its DMA queue.
All transfers are plain 2D/3D slices: the host pre-permutes IFadj and
adj into pair-interleaved layouts, W2 partition-major, and the z
bounce is written p-major so gathered z is plain-sliceable.

Precision: layer-1 runs lhsT=S in bf16 against the IFadj moving
operand in fp8e4, CENTERED: B = IFadj - 1/2 (entries U[0,1] ->
[-1/2,1/2]); the exact mean term 1/2*colsum(S)_j is folded into the
layer-1 bias on the host (colsum(S) = (sum_m x[m,:]) @ W1, a trivial
host matvec). Measured end-to-end rel err ~2.2e-3 vs the 2e-2 gate.
The layer-2 spmm runs both operands fp8e4 with z pre-scaled by 1/4 to
sit far inside e4m3 range.
"""

import numpy as np
import ml_dtypes

NCORES = 8
N = 8192
NFEAT = 1024
NHID = 512
OUTD = 256
ROWS = N // NCORES  # 1024
P = 128
BN_EPS = 1e-5

CB = NFEAT // P   # 8  c-blocks (x feature contraction)
IB = ROWS // P    # 8  i-blocks per node block
JB = NHID // P    # 4  j-blocks (hidden)
MT = N // P       # 64 m-tiles (global node contraction)
HF = 512          # matmul moving free dim (PSUM bank limit)
IH = ROWS // HF   # 2 i-halves of the local row range
OB = OUTD // P    # 2 output-feature blocks
GC = 2            # allgather chunks (S and Z)
QT = 4            # m-tiles per (chunk, block) quarter
NRED = 3          # redundant S blocks (global blocks 5..7)
RED0 = NCORES - NRED  # first redundant block = 5

_BF16 = ml_dtypes.bfloat16
_F8 = ml_dtypes.float8_e4m3

_cache = {}


def _build():
    import concourse.mybir as mybir
    import concourse.tile as tile
    from concourse import bacc

    dt = mybir.dt
    f32 = dt.float32
    bf16 = dt.bfloat16
    f8 = dt.float8e4
    AF = mybir.ActivationFunctionType
    DR = mybir.MatmulPerfMode.DoubleRow
    MULT = mybir.AluOpType.mult

    nc = bacc.Bacc("TRN2", target_bir_lowering=False, debug=False,
                   num_devices=NCORES)

    xT_e = nc.dram_tensor("xT", [NFEAT, ROWS], bf16, kind="ExternalInput")
    # replicated x rows for global node blocks 5..7 (same on every core)
    xTr_e = nc.dram_tensor("xTr", [NFEAT, NRED * ROWS], bf16,
                           kind="ExternalInput")
    # centered IFadj^T in fp8, pair-interleaved and split by column
    # half on the host: row ih*4096 + pair*P + p, col t*HF + c holds
    # IFadjT[(2*pair+t)*P + p, ih*HF + c] - 1/2
    ifadjH_e = nc.dram_tensor("ifadjH", [IH * N // 2, 2 * HF], f8,
                              kind="ExternalInput")
    # adj rows pair-interleaved on host: row pair*P+p holds m-tiles
    # (2*pair, 2*pair+1) side by side -> [P, 2, ROWS] is a plain slice
    adjP_e = nc.dram_tensor("adjP", [N // 2, 2 * ROWS], f8,
                            kind="ExternalInput")
    w1_e = nc.dram_tensor("w1", [NFEAT, NHID], bf16, kind="ExternalInput")
    # W2/4, partition-major: [P, JB*OUTD]
    w2_e = nc.dram_tensor("w2", [P, JB * OUTD], bf16, kind="ExternalInput")
    # layer-1 bias + 1/2*colsum(S) fold, [P, JB]
    b1p_e = nc.dram_tensor("b1p", [P, JB], f32, kind="ExternalInput")
    bnsc_e = nc.dram_tensor("bnsc", [P, OB], f32, kind="ExternalInput")
    bnbi_e = nc.dram_tensor("bnbi", [P, OB], f32, kind="ExternalInput")
    # outT: [OUTD, ROWS]; the host transposes each core's block.
    out_e = nc.dram_tensor("out", [OUTD, ROWS], f32, kind="ExternalOutput")

    groups = [list(range(NCORES))]

    def allgather(g_in, g_out):
        nc.gpsimd.collective_compute(
            "AllGather", mybir.AluOpType.bypass, replica_groups=groups,
            ins=[g_in[:]], outs=[g_out[:]])

    with tile.TileContext(nc) as tc:
        with (
            tc.tile_pool(name="const", bufs=1) as const,
            tc.tile_pool(name="xslab", bufs=8) as xslab_p,
            tc.tile_pool(name="sloc", bufs=1) as sloc_p,
            tc.tile_pool(name="sred", bufs=1) as sred_p,
            tc.tile_pool(name="sgt", bufs=(NCORES - NRED) * IB) as sgt_p,
            tc.tile_pool(name="h1", bufs=1) as h1_p,
            tc.tile_pool(name="zsb", bufs=1) as z_p,
            tc.tile_pool(name="zchunk", bufs=8) as zchunk_p,
            tc.tile_pool(name="astream", bufs=12) as astream,
            tc.tile_pool(name="apair", bufs=8) as apair_p,
            tc.tile_pool(name="outsb", bufs=1) as outsb_p,
            tc.tile_pool(name="dram", bufs=1, space="DRAM") as dram,
        ):
            # ---- DRAM bounce buffers for the collectives
            RPC = ROWS // GC  # rows bounced per S chunk (512)
            s_bounce = [dram.tile([RPC, NHID], bf16, name=f"sb{c}")
                        for c in range(GC)]
            # s_all[c] row k*RPC + r = S[global row k*ROWS + c*RPC + r]
            s_all = [dram.tile([RPC * NCORES, NHID], bf16,
                               addr_space="Shared", name=f"sa{c}")
                     for c in range(GC)]
            # z bounce is p-major: row p holds (t, o), t = chunk-local
            # i-block -> gathered z is plain-sliceable per core block
            z_bounce = [dram.tile([P, QT * OUTD], f8, name=f"zb{c}")
                        for c in range(GC)]
            z_all = [dram.tile([P * NCORES, QT * OUTD], f8,
                               addr_space="Shared", name=f"za{c}")
                     for c in range(GC)]

            s_loc = sloc_p.tile([P, IB, NHID], bf16)
            s_red = sred_p.tile([P, NRED * IB, NHID], bf16)
            w1_sb = const.tile([P, CB, NHID], bf16)

            # ---- phase A: own S block in two i-half sub-passes so the
            # first AllGather chunk fires at ~half-A; x/W1 slab loads
            # interleaved so the first matmul starts as soon as (w1
            # slab 0, x slab 0) land. Triggers at top priority.
            with tc.tile_pool(name="psA", bufs=1, space="PSUM") as psA:
                xs_slabs = []
                for cb in range(CB):
                    nc.sync.dma_start(w1_sb[:, cb, :],
                                      w1_e[cb * P:(cb + 1) * P, :])
                    xs = xslab_p.tile([P, ROWS], bf16, tag="xslab")
                    nc.sync.dma_start(xs[:], xT_e[cb * P:(cb + 1) * P, :])
                    xs_slabs.append(xs)
                for c in range(GC):
                    ps_own = [psA.tile([P, NHID], f32, name=f"pso{c}_{t}",
                                       tag=f"pa{t}")
                              for t in range(IB // GC)]
                    for cb in range(CB):
                        for t in range(IB // GC):
                            ib = c * (IB // GC) + t
                            nc.tensor.matmul(
                                ps_own[t][:],
                                xs_slabs[cb][:, ib * P:(ib + 1) * P],
                                w1_sb[:, cb, :],
                                start=(cb == 0), stop=(cb == CB - 1))
                    for t in range(IB // GC):
                        ib = c * (IB // GC) + t
                        nc.vector.tensor_scalar(
                            s_loc[:, ib, :], ps_own[t][:], 1.0, None,
                            MULT)
                        nc.sync.dma_start(
                            s_bounce[c][t * P:(t + 1) * P, :],
                            s_loc[:, ib, :])
                    with tc.high_priority():
                        allgather(s_bounce[c], s_all[c])

                # remaining constants (needed from phase C on)
                b1p_sb = const.tile([P, JB], f32)
                nc.sync.dma_start(b1p_sb[:], b1p_e[:])
                w2_sb = const.tile([P, JB, OUTD], bf16)
                nc.sync.dma_start(w2_sb[:], w2_e[:])
                bnsc_sb = const.tile([P, OB], f32)
                nc.sync.dma_start(bnsc_sb[:], bnsc_e[:])
                bnbi_sb = const.tile([P, OB], f32)
                nc.sync.dma_start(bnbi_sb[:], bnbi_e[:])

                # ---- phase B: redundant S for global blocks 5..7
                for r in range(NRED):
                    ps_r = [psA.tile([P, NHID], f32, name=f"psr{r}_{ib}",
                                     tag=f"pa{ib}")
                            for ib in range(IB)]
                    for cb in range(CB):
                        xs = xslab_p.tile([P, ROWS], bf16, tag="xslab")
                        nc.sync.dma_start(
                            xs[:],
                            xTr_e[cb * P:(cb + 1) * P,
                                  r * ROWS:(r + 1) * ROWS])
                        for ib in range(IB):
                            nc.tensor.matmul(
                                ps_r[ib][:], xs[:, ib * P:(ib + 1) * P],
                                w1_sb[:, cb, :],
                                start=(cb == 0), stop=(cb == CB - 1))
                    for ib in range(IB):
                        nc.vector.tensor_scalar(
                            s_red[:, r * IB + ib, :], ps_r[ib][:], 1.0,
                            None, MULT)

            h1T = h1_p.tile([P, JB, ROWS], bf16)
            z_sb = z_p.tile([P, IB, OUTD], f8)
            s_gt = {}

            # m-traversal per pass, in adjacent PAIRS (one IFadj dual
            # tile feeds two m-tiles): local blocks 5..7 first, then
            # gathered blocks chunk-major.
            pair_walk = ([(g, q) for g in range(RED0, NCORES)
                          for q in range(0, IB, 2)]
                         + [(g, c * QT + t) for c in range(GC)
                            for g in range(RED0)
                            for t in range(0, QT, 2)])

            # ---- phase C, i-half pass ih. Streaming dual tiles on the
            # Activation queue; gathered-S staging on the gpsimd DGE.
            def l1_pass(ih, psh, psz):
                psum_h = [psh.tile([P, HF], f32, name=f"ph{jb}_{ih}",
                                   tag=f"ph{jb}")
                          for jb in range(JB)]
                n_emitted = 0
                for g, q in pair_walk:
                    a_dual = astream.tile([P, 2, HF], f8, tag="adual")
                    pidx = (g * IB + q) // 2
                    nc.scalar.dma_start(
                        a_dual[:],
                        ifadjH_e[ih * (N // 2) + pidx * P:
                                 ih * (N // 2) + (pidx + 1) * P, :])
                    for u in range(2):
                        qq = q + u
                        if g >= RED0:
                            s_src = s_red[:, (g - RED0) * IB + qq, :]
                        else:
                            if ih == 0 and s_gt.get((g, qq)) is None:
                                c, t = divmod(qq, QT)
                                st = sgt_p.tile([P, NHID], bf16,
                                                name=f"sg{g}_{qq}",
                                                tag="sgt")
                                nc.gpsimd.dma_start(
                                    st[:],
                                    s_all[c][(g * QT + t) * P:
                                             (g * QT + t + 1) * P, :])
                                s_gt[(g, qq)] = st
                            s_src = s_gt[(g, qq)][:]
                        for jb in range(JB):
                            nc.tensor.matmul(
                                psum_h[jb][:],
                                s_src[:, jb * P:(jb + 1) * P],
                                a_dual[:, u, :],
                                start=(n_emitted == 0),
                                stop=(n_emitted == MT - 1),
                            )
                        n_emitted += 1
                # epilogue: relu + (b1 + colsum/2) bias into h1T half
                for jb in range(JB):
                    nc.scalar.activation(
                        h1T[:, jb, ih * HF:(ih + 1) * HF],
                        psum_h[jb][:], AF.Relu,
                        bias=b1p_sb[:, jb:jb + 1])
                # z for this half's i-blocks (fp8, W2 pre-scaled by 1/4),
                # p-major bounce, gather chunk ih
                for t in range(IB // IH):
                    ib = ih * (IB // IH) + t
                    ps = psz.tile([P, OUTD], f32, tag="z")
                    for jb in range(JB):
                        nc.tensor.matmul(
                            ps[:],
                            h1T[:, jb, ib * P:(ib + 1) * P],
                            w2_sb[:, jb, :],
                            start=(jb == 0), stop=(jb == JB - 1),
                        )
                    nc.vector.tensor_scalar(
                        z_sb[:, ib, :], ps[:], 1.0, None, MULT)
                    nc.sync.dma_start(
                        z_bounce[ih][:, t * OUTD:(t + 1) * OUTD],
                        z_sb[:, ib, :])
                allgather(z_bounce[ih], z_all[ih])

            with (
                tc.tile_pool(name="psh", bufs=1, space="PSUM") as psh,
                tc.tile_pool(name="psz", bufs=2, space="PSUM") as psz,
            ):
                for ih in range(IH):
                    l1_pass(ih, psh, psz)

            # ---- phase D: outT[o, i] = sum_m Z[m, o] * adjT[m, i]
            # fp8 DoubleRow, one matmul per adjacent m-tile pair.
            # z_all[c] row k*P+p holds (t, o) = z[k*ROWS + c*RPC + t*P+p]
            # -> m-tile of (c, k, t) is 8k + 4c + t.
            outT_sb = outsb_p.tile([P, OB, ROWS], f32)
            with tc.tile_pool(name="ps4", bufs=1, space="PSUM") as ps4:
                psum_o = [[ps4.tile([P, HF], f32, name=f"po{ob}_{ih}",
                                    tag=f"po{ob}_{ih}")
                           for ih in range(IH)] for ob in range(OB)]
                first = True
                for c in range(GC):
                    for k in range(NCORES):
                        zc_sb = zchunk_p.tile([P, QT, OUTD], f8,
                                              tag="zchunk")
                        nc.gpsimd.dma_start(
                            zc_sb[:], z_all[c][k * P:(k + 1) * P, :])
                        for pr in range(0, QT, 2):
                            mt = IB * k + QT * c + pr
                            a_pair = apair_p.tile([P, 2, ROWS], f8,
                                                  tag="apair")
                            nc.scalar.dma_start(
                                a_pair[:],
                                adjP_e[(mt // 2) * P:(mt // 2 + 1) * P, :])
                            last_pr = (c == GC - 1 and k == NCORES - 1
                                       and pr == QT - 2)
                            for ob in range(OB):
                                for ih in range(IH):
                                    nc.tensor.matmul(
                                        psum_o[ob][ih][:],
                                        zc_sb[:, pr:pr + 2,
                                              ob * P:(ob + 1) * P],
                                        a_pair[:, :,
                                               ih * HF:(ih + 1) * HF],
                                        start=first, stop=last_pr,
                                        perf_mode=DR,
                                    )
                            first = False
                # fused BN affine on PSUM evict: out = psum*scale + bias
                for ob in range(OB):
                    for ih in range(IH):
                        nc.vector.tensor_scalar(
                            outT_sb[:, ob, ih * HF:(ih + 1) * HF],
                            psum_o[ob][ih][:],
                            bnsc_sb[:, ob:ob + 1],
                            bnbi_sb[:, ob:ob + 1],
                            mybir.AluOpType.mult,
                            mybir.AluOpType.add)
                    nc.sync.dma_start(
                        out_e[ob * P:(ob + 1) * P, :], outT_sb[:, ob, :])

    nc.compile()
    return nc


def _get_nc():
    if "nc" not in _cache:
        _cache["nc"] = _build()
    return _cache["nc"]


def kernel(x, IFadj, adj, W1, b1, W2, b2, bn_gamma, bn_beta, bn_mean, bn_var):
    from concourse.bass_utils import run_bass_kernel_spmd

    x = np.asarray(x, dtype=np.float32)
    IFadj = np.asarray(IFadj, dtype=np.float32)
    adj = np.asarray(adj, dtype=np.float32)
    W1 = np.asarray(W1, dtype=np.float32)
    b1 = np.asarray(b1, dtype=np.float32)
    W2 = np.asarray(W2, dtype=np.float32)
    b2 = np.asarray(b2, dtype=np.float32)
    bn_gamma = np.asarray(bn_gamma, dtype=np.float32)
    bn_beta = np.asarray(bn_beta, dtype=np.float32)
    bn_mean = np.asarray(bn_mean, dtype=np.float32)
    bn_var = np.asarray(bn_var, dtype=np.float32)

    # host-side prep: shard rows, transpose for PE lhsT layout, cast.
    # W2 is pre-scaled by 1/4 so z stays well inside fp8e4 range; the
    # BN scale is multiplied by 4 to undo it after the layer-2 spmm.
    w1b = W1.astype(_BF16)
    w2b = np.ascontiguousarray(
        (W2 * 0.25).astype(_BF16).reshape(JB, P, OUTD)
        .transpose(1, 0, 2).reshape(P, JB * OUTD))
    # layer-1 bias including the exact 1/2*colsum(S) centering term
    colsum = x.sum(axis=0, dtype=np.float64).astype(np.float32) @ W1
    b1c = b1 + 0.5 * colsum
    b1p = np.ascontiguousarray(b1c.reshape(JB, P).T)  # [P, JB]
    inv = bn_gamma / np.sqrt(bn_var + BN_EPS)
    bias_tot = b2 * inv + bn_beta - bn_mean * inv
    bnsc = np.ascontiguousarray((4.0 * inv).reshape(OB, P).T)   # [P, OB]
    bnbi = np.ascontiguousarray(bias_tot.reshape(OB, P).T)      # [P, OB]

    # replicated x rows for global node blocks 5..7
    xTr = np.ascontiguousarray(x[RED0 * ROWS:].T).astype(_BF16)

    in_maps = []
    for k in range(NCORES):
        r0, r1 = k * ROWS, (k + 1) * ROWS
        # centered IFadj^T in fp8: [m, col] -> [ih, pair, p, t, c]
        A8 = (IFadj[r0:r1].T - np.float32(0.5)).astype(_F8)  # [N, ROWS]
        ifadjH = np.ascontiguousarray(
            A8.reshape(N // 256, 2, P, IH, HF).transpose(3, 0, 2, 1, 4)
            .reshape(IH * N // 2, 2 * HF))
        adjT8 = np.ascontiguousarray(adj[r0:r1].T).astype(_F8)  # [N, ROWS]
        # pair-interleave: row pair*P+p = m-tiles (2p, 2p+1) side by side
        adjP = np.ascontiguousarray(
            adjT8.reshape(N // 256, 2, P, ROWS).transpose(0, 2, 1, 3)
            .reshape(N // 2, 2 * ROWS))
        in_maps.append({
            "xT": np.ascontiguousarray(x[r0:r1].T).astype(_BF16),
            "xTr": xTr,
            "ifadjH": ifadjH,
            "adjP": adjP,
            "w1": w1b,
            "w2": w2b,
            "b1p": b1p,
            "bnsc": bnsc,
            "bnbi": bnbi,
        })

    global _last_in_maps
    _last_in_maps = in_maps

    nc = _get_nc()
    try:
        res = run_bass_kernel_spmd(nc, in_maps, list(range(NCORES)))
    except Exception:
        # transient device wedge (NRT_EXEC_UNIT_UNRECOVERABLE etc.) --
        # a straight retry has been observed to recover
        import time
        time.sleep(2.0)
        res = run_bass_kernel_spmd(nc, in_maps, list(range(NCORES)))
    # per-core output is outT [OUTD, ROWS]; transpose back and stack rows
    return np.concatenate(
        [np.ascontiguousarray(res.results[k]["out"].T)
         for k in range(NCORES)], axis=0)
